# revision 26
# baseline (speedup 1.0000x reference)
"""Trainium2 Bass kernel for nn_ExampleModel_1116691497724 (moe_routing).

Math: the reference returns log_softmax_T( sum_D(moe_out) ), and sum_D
collapses the expert FFN to a dot product:
    sum_d (h @ W2[e] + b2[e]) = h . w2sum[e] + sum(b2[e]),  w2sum[e] = W2[e] @ 1
    (x @ W1[e] + b1[e]) . w2sum[e] = x . v[e] + c[e]
with v[e] = W1[e] @ w2sum[e]  (a [D] vector) and scalar
c[e] = b1[e].w2sum[e] + sum(b2[e]).  Then per token:
    s_e = x . v[e] + c[e],  logits = x @ Wg
    moe_sum = max(softmax(logits)) * s_argmax(logits)
    out = log_softmax over tokens (per batch row) of moe_sum.

Distribution over 8 cores, two launches (measured: a single ncfw collective
costs ~65us of barrier/trigger latency on this runtime — far more than a
second launch's fixed ~17us, so the 16KB cross-core combine happens on the
host between launches; the host does only that partial sum, all real math
stays on device):
  launch A (expert-parallel over H): core c reduces W2[:, 128c:128c+128, :]
    and computes partial v from the matching W1 columns (f32r stream after a
    rounding pass) -> outputs [v0 | v1 | c0 c1] partials (16KB); host sums.
  launch B (token-parallel): core c owns batch row c%4 (512 tokens): logits
    stream in fp32 (exact — argmax ties must match the reference), s stream
    in f32r, gate/select per token after a PE transpose, row log_softmax via
    PE transposes (no cross-partition DMA).  Host takes rows from cores 0..3.

Scheduling: stationary matmul operands are tiny (M<=4) so LDWEIGHTS is
negligible; fp32 streams at 4 cycles/row, f32r at 1.  Big loads alternate the
two HWDGE rings (SP via nc.sync, ACT via nc.scalar) for concurrency.  The d
axis is decomposed as d = p*16 + n so the flat v vector loads into [128,16]
tiles with contiguous per-partition runs.
"""

import sys

import numpy as np

for _p in ("/opt/trn_rl_repo",):
    if _p not in sys.path:
        sys.path.append(_p)

import concourse.bass as bass  # noqa: E402
import concourse.mybir as mybir  # noqa: E402
import concourse.tile as tile  # noqa: E402
from concourse import bacc, bass_utils  # noqa: E402
from concourse.masks import make_identity  # noqa: E402

# Problem shape (hardcoded per spec).
B, T, D, H, E = 4, 512, 2048, 1024, 2
P = 128
NCORES = 8
TB = T  # tokens per core = one batch row
NB = D // P  # 16 d-blocks
HC = H // NCORES  # 128 h-chunk per expert per core
NG = TB // P  # 4 token groups per core
DC = D // NCORES  # 256 b2 columns per core
VK = 4  # v computed in VK chunks of D/VK columns
F32 = mybir.dt.float32
F32R = mybir.dt.float32r
AX = mybir.AxisListType
AF = mybir.ActivationFunctionType
ALU = mybir.AluOpType

VPART = 2 * D + 2  # launch A output: v0 | v1 | c0 c1


def emit_phase_a(nc, tc, io):
    """w2sum + partial v for this core's H-chunk -> vpart [1, 2D+2]."""
    w1t, w2r, b1c, b2c, vout = io["w1t"], io["w2r"], io["b1c"], io["b2c"], io["vout"]
    with (
        tc.tile_pool(name="main", bufs=1) as pool,
        tc.tile_pool(name="psum", bufs=1, space="PSUM") as psum,
    ):
        # three DMA queues (SP ring, ACT ring, SWDGE) ~140GB/s each; W2 halves
        # first (they gate the reduce), W1 spread over all three
        b1_sb = pool.tile([P, E], F32)
        nc.gpsimd.dma_start(b1_sb[:], b1c)
        b2_sb = pool.tile([1, E * DC], F32)
        nc.gpsimd.dma_start(b2_sb[:], b2c)
        HD = D // 2
        w2_sb = pool.tile([P, E, D], F32)
        for h in range(2):
            nc.sync.dma_start(w2_sb[:, 0, h * HD : (h + 1) * HD], w2r[0, :, h * HD : (h + 1) * HD])
            nc.scalar.dma_start(w2_sb[:, 1, h * HD : (h + 1) * HD], w2r[1, :, h * HD : (h + 1) * HD])
        # W1 goes straight into an f32r tile (w1t is declared float32r; same
        # bits, marks the rounding for the PE) — no cast pass needed.  Only
        # the two HW rings: the SWDGE queue is slow (~90GB/s measured)
        w1r = pool.tile([P, E, D], F32R)
        for h in range(2):
            nc.sync.dma_start(w1r[:, 0, h * HD : (h + 1) * HD], w1t[0, :, h * HD : (h + 1) * HD])
            nc.scalar.dma_start(w1r[:, 1, h * HD : (h + 1) * HD], w1t[1, :, h * HD : (h + 1) * HD])

        w2h = pool.tile([P, 2 * E], F32)
        w2s = pool.tile([P, E], F32)
        for e in range(E):
            for h in range(2):
                nc.vector.reduce_sum(
                    w2h[:, 2 * e + h : 2 * e + h + 1],
                    w2_sb[:, e, h * HD : (h + 1) * HD],
                    axis=AX.X,
                )
            nc.vector.tensor_add(
                w2s[:, e : e + 1], w2h[:, 2 * e : 2 * e + 1], w2h[:, 2 * e + 1 : 2 * e + 2]
            )
        w2s_r = pool.tile([P, E], F32R)
        nc.vector.tensor_copy(w2s_r[:], w2s[:])
        b2s = pool.tile([1, E], F32)
        for e in range(E):
            nc.vector.reduce_sum(
                b2s[0:1, e : e + 1], b2_sb[0:1, e * DC : (e + 1) * DC], axis=AX.X
            )

        pay = pool.tile([1, VPART], F32)
        b1dot = psum.tile([1, E], F32)
        DK = D // VK
        for e in range(E):
            for k in range(VK):
                vch = psum.tile([1, DK], F32, name="vch", tag="vch", bufs=2)
                nc.tensor.matmul(
                    vch[:],
                    w2s_r[:, e : e + 1],
                    w1r[:, e, k * DK : (k + 1) * DK],
                    start=True,
                    stop=True,
                )
                dst = pay[0:1, e * D + k * DK : e * D + (k + 1) * DK]
                if k % 2 == 0:
                    nc.vector.tensor_copy(dst, vch[:])
                else:
                    nc.scalar.copy(dst, vch[:])
            nc.tensor.matmul(
                b1dot[0:1, e : e + 1],
                w2s[:, e : e + 1],
                b1_sb[:, e : e + 1],
                start=True,
                stop=True,
            )
            nc.vector.tensor_add(
                pay[0:1, 2 * D + e : 2 * D + e + 1],
                b1dot[0:1, e : e + 1],
                b2s[0:1, e : e + 1],
            )
        nc.sync.dma_start(vout[:], pay[:])


def emit_phase_b(nc, tc, io):
    """logits (fp32) + s (f32r) streams, gate/select, row log_softmax."""
    xt, wgt, vin, out = io["xt"], io["wgt"], io["vin"], io["out"]
    rings = [nc.sync, nc.scalar]
    with (
        tc.tile_pool(name="main", bufs=1) as pool,
        tc.tile_pool(name="psum", bufs=1, space="PSUM") as psum,
    ):
        # tiny loads first, then the 4MB x load chunked over both rings
        wgt_sb = pool.tile([P, NB * E], F32)
        nc.sync.dma_start(wgt_sb[:], wgt)
        wg3 = wgt_sb.rearrange("p (n e) -> p n e", e=E)
        vsb = pool.tile([P, E, NB], F32)
        for e in range(E):
            nc.scalar.dma_start(
                vsb[:, e, :],
                vin[0:1, e * D : (e + 1) * D].rearrange("x (p n) -> p (x n)", p=P),
            )
        csum = pool.tile([1, E], F32)
        nc.sync.dma_start(csum[:], vin[0:1, 2 * D : 2 * D + E])

        # x over three DMA queues, graduated chunk sizes so the PE streams
        # start on the first block ASAP; matmuls follow arrival order
        x_sb = pool.tile([P, NB, TB], F32)
        xv = xt.rearrange("(p n) t -> p n t", p=P)
        qs = [nc.sync, nc.scalar, nc.gpsimd]
        chunks = [
            (0, 0, 1), (1, 1, 2), (2, 2, 3),
            (0, 3, 5), (1, 5, 7), (2, 7, 9),
            (0, 9, 12), (1, 12, 15), (2, 15, 16),
        ]
        for q, lo, hi in chunks:
            qs[q].dma_start(x_sb[:, lo:hi, :], xv[:, lo:hi, :])

        # preload ACT tables (Exp, Ln) off the critical path; keep ALL copy
        # work off the scalar engine so these tables are never evicted
        warm = pool.tile([1, 2], F32)
        nc.gpsimd.memset(warm[:], 1.0)
        wz = pool.tile([1, 2], F32)
        nc.scalar.activation(wz[:], warm[:], AF.Exp)
        nc.scalar.activation(wz[:], warm[:], AF.Ln)

        # small f32r prep FIRST (DVE is FIFO — these must not queue behind the
        # 4MB of x casts), then the x cast chain trailing the DMA chunks
        vsb_r = pool.tile([P, E, NB], F32R)
        nc.vector.tensor_copy(vsb_r[:], vsb[:])
        csum_b = pool.tile([P, E], F32)
        nc.gpsimd.partition_broadcast(csum_b[:], csum[0:1, :])
        x_r = pool.tile([P, NB, TB], F32R)
        for _, lo, hi in chunks:
            nc.vector.tensor_copy(x_r[:, lo:hi, :], x_sb[:, lo:hi, :])

        # logits (fp32 exact) and s (f32r) streams, interleaved per block in
        # expected DMA arrival order
        lg_ps = psum.tile([E, TB], F32)
        sg_ps = psum.tile([E, TB], F32)
        order = []
        heads = [list(range(lo, hi)) for _, lo, hi in chunks]
        for trio in range(3):
            for blocks in heads[3 * trio : 3 * trio + 3]:
                order.extend(blocks)
        for i, n in enumerate(order):
            nc.tensor.matmul(
                lg_ps[:], wg3[:, n, :], x_sb[:, n, :], start=(i == 0), stop=(i == NB - 1)
            )
            nc.tensor.matmul(
                sg_ps[:], vsb_r[:, :, n], x_r[:, n, :], start=(i == 0), stop=(i == NB - 1)
            )
        sbl = pool.tile([E, TB], F32)
        nc.vector.tensor_copy(sbl[:], lg_ps[:])
        ident = pool.tile([P, P], F32)
        make_identity(nc, ident[:])

        gates, masks = [], []
        for g in range(NG):
            tpl = psum.tile([P, E], F32, name=f"tpl_{g}", tag="tp", bufs=2)
            nc.tensor.transpose(tpl[:], sbl[0:E, g * P : (g + 1) * P], ident[0:E, 0:E])
            t2l = pool.tile([P, E], F32, name=f"t2l_{g}")
            nc.vector.tensor_copy(t2l[:], tpl[:])
            negm = pool.tile([P, 1], F32, name=f"negm_{g}")
            nc.vector.reduce_max(negm[:], t2l[:], axis=AX.X, negate=True)
            z = pool.tile([P, E], F32, name=f"z_{g}")
            den = pool.tile([P, 1], F32, name=f"den_{g}")
            nc.scalar.activation(z[:], t2l[:], AF.Exp, bias=negm[:], accum_out=den[:])
            rec = pool.tile([P, 1], F32, name=f"rec_{g}")
            nc.vector.reciprocal(rec[:], den[:])
            zmax = pool.tile([P, 1], F32, name=f"zmax_{g}")
            nc.vector.reduce_max(zmax[:], z[:], axis=AX.X)
            gate = pool.tile([P, 1], F32, name=f"gate_{g}")
            nc.vector.tensor_mul(gate[:], zmax[:], rec[:])
            mask = pool.tile([P, 1], F32, name=f"mask_{g}")
            nc.vector.tensor_tensor(mask[:], t2l[:, 0:1], t2l[:, 1:2], op=ALU.is_ge)
            gates.append(gate)
            masks.append(mask)

        sbs = pool.tile([E, TB], F32)
        nc.vector.tensor_copy(sbs[:], sg_ps[:])

        moe_sb = pool.tile([P, NG], F32)
        for g in range(NG):
            tps = psum.tile([P, E], F32, name=f"tps_{g}", tag="tp", bufs=2)
            nc.tensor.transpose(tps[:], sbs[0:E, g * P : (g + 1) * P], ident[0:E, 0:E])
            t2s = pool.tile([P, E], F32, name=f"t2s_{g}")
            nc.vector.tensor_add(t2s[:], tps[:], csum_b[:])
            sdiff = pool.tile([P, 1], F32, name=f"sdiff_{g}")
            nc.vector.tensor_sub(sdiff[:], t2s[:, 0:1], t2s[:, 1:2])
            ssel = pool.tile([P, 1], F32, name=f"ssel_{g}")
            nc.vector.tensor_mul(ssel[:], masks[g][:], sdiff[:])
            nc.vector.tensor_add(ssel[:], ssel[:], t2s[:, 1:2])
            nc.vector.tensor_mul(moe_sb[:, g : g + 1], gates[g][:], ssel[:])

        # row log_softmax over all 512 tokens, via PE transposes
        tp4 = psum.tile([NG, P], F32)
        nc.tensor.transpose(tp4[:], moe_sb[:], ident[:])
        sb4t = pool.tile([NG, P], F32)
        nc.vector.tensor_copy(sb4t[:], tp4[:])
        m4p = pool.tile([NG, 1], F32)
        nc.vector.reduce_max(m4p[:], sb4t[:], axis=AX.X)
        m1p = psum.tile([1, NG], F32, name="m1p", tag="t1", bufs=2)
        nc.tensor.transpose(m1p[:], m4p[:], ident[0:NG, 0:NG])
        negm2 = pool.tile([1, 1], F32)
        nc.vector.reduce_max(negm2[:], m1p[:], axis=AX.X, negate=True)
        negm4 = pool.tile([NG, 1], F32)
        nc.gpsimd.partition_broadcast(negm4[:], negm2[:])
        e4 = pool.tile([NG, P], F32)
        s4 = pool.tile([NG, 1], F32)
        nc.scalar.activation(e4[:], sb4t[:], AF.Exp, bias=negm4[:], accum_out=s4[:])
        s1p = psum.tile([1, NG], F32, name="s1p", tag="t1", bufs=2)
        nc.tensor.transpose(s1p[:], s4[:], ident[0:NG, 0:NG])
        ssum = pool.tile([1, 1], F32)
        nc.vector.reduce_sum(ssum[:], s1p[:], axis=AX.X)
        logs = pool.tile([1, 1], F32)
        nc.scalar.activation(logs[:], ssum[:], AF.Ln)
        shift = pool.tile([1, 1], F32)
        nc.vector.tensor_sub(shift[:], negm2[:], logs[:])
        shift4 = pool.tile([NG, 1], F32)
        nc.gpsimd.partition_broadcast(shift4[:], shift[:])
        res4 = pool.tile([NG, P], F32)
        nc.vector.tensor_scalar_add(res4[:], sb4t[:], shift4[:])
        nc.sync.dma_start(out.rearrange("x (g p) -> g (x p)", p=P), res4[:])


_CACHED = {}


def build_program(which):
    if which in _CACHED:
        return _CACHED[which]
    nc = bacc.Bacc(
        "TRN2",
        target_bir_lowering=False,
        debug=False,
        enable_asserts=False,
        num_devices=NCORES,
    )
    if which == "a":
        io = {
            "w1t": nc.dram_tensor("w1t", [E, HC, D], F32R, kind="ExternalInput").ap(),
            "w2r": nc.dram_tensor("w2r", [E, HC, D], F32, kind="ExternalInput").ap(),
            "b1c": nc.dram_tensor("b1c", [HC, E], F32, kind="ExternalInput").ap(),
            "b2c": nc.dram_tensor("b2c", [1, E * DC], F32, kind="ExternalInput").ap(),
            "vout": nc.dram_tensor("vout", [1, VPART], F32, kind="ExternalOutput").ap(),
        }
        emit = emit_phase_a
    else:
        io = {
            "xt": nc.dram_tensor("xt", [D, TB], F32, kind="ExternalInput").ap(),
            "wgt": nc.dram_tensor("wgt", [P, NB * E], F32, kind="ExternalInput").ap(),
            "vin": nc.dram_tensor("vin", [1, VPART], F32, kind="ExternalInput").ap(),
            "out": nc.dram_tensor("out", [1, TB], F32, kind="ExternalOutput").ap(),
        }
        emit = emit_phase_b
    with tile.TileContext(nc) as tc:
        emit(nc, tc, io)
    nc.compile()
    _CACHED[which] = nc
    return nc


def shard_inputs_a(Wg, W1, b1, W2, b2):
    W1 = np.asarray(W1, np.float32)
    b1 = np.asarray(b1, np.float32)
    W2 = np.asarray(W2, np.float32)
    b2 = np.asarray(b2, np.float32)
    in_maps = []
    for c in range(NCORES):
        hs, he = c * HC, (c + 1) * HC
        in_maps.append(
            {
                "w1t": np.ascontiguousarray(W1[:, :, hs:he].transpose(0, 2, 1)),
                "w2r": np.ascontiguousarray(W2[:, hs:he, :]),
                "b1c": np.ascontiguousarray(b1[:, hs:he].T),
                "b2c": np.ascontiguousarray(
                    b2[:, c * DC : (c + 1) * DC].reshape(1, E * DC)
                ),
            }
        )
    return in_maps


def shard_inputs_b(x, Wg, vpart_sum):
    x = np.asarray(x, np.float32).reshape(B * T, D)
    Wg = np.asarray(Wg, np.float32)
    # wgt[p, n*2+e] = Wg[p*16+n, e]  (d = p*16 + n decomposition)
    wgt = np.ascontiguousarray(Wg.reshape(P, NB * E))
    in_maps = []
    for c in range(NCORES):
        row = c % B
        in_maps.append(
            {
                "xt": np.ascontiguousarray(x[row * TB : (row + 1) * TB, :].T),
                "wgt": wgt,
                "vin": vpart_sum,
            }
        )
    return in_maps


def run_a(in_maps, **kwargs):
    return bass_utils.run_bass_kernel_spmd(
        build_program("a"), in_maps, core_ids=list(range(NCORES)), **kwargs
    )


def run_b(in_maps, **kwargs):
    return bass_utils.run_bass_kernel_spmd(
        build_program("b"), in_maps, core_ids=list(range(NCORES)), **kwargs
    )


def kernel(x, Wg, W1, b1, W2, b2):
    res_a = run_a(shard_inputs_a(Wg, W1, b1, W2, b2))
    # cross-core combine: sum of the 8 per-core partials (the gather/reshard
    # step between the two launches; 16KB, no model math beyond the reduction)
    vpart = np.sum([res_a.results[c]["vout"] for c in range(NCORES)], axis=0)
    vpart = np.ascontiguousarray(vpart, np.float32)
    res_b = run_b(shard_inputs_b(x, Wg, vpart))
    return np.concatenate([res_b.results[b]["out"] for b in range(B)], axis=0)


# revision 32
# speedup vs baseline: 1.0564x; 1.0564x over previous
"""Trainium2 Bass kernel for nn_ExampleModel_1116691497724 (moe_routing).

Math: the reference returns log_softmax_T( sum_D(moe_out) ), and sum_D
collapses the expert FFN to a dot product:
    sum_d (h @ W2[e] + b2[e]) = h . w2sum[e] + sum(b2[e]),  w2sum[e] = W2[e] @ 1
    (x @ W1[e] + b1[e]) . w2sum[e] = x . v[e] + c[e]
with v[e] = W1[e] @ w2sum[e]  (a [D] vector) and scalar
c[e] = b1[e].w2sum[e] + sum(b2[e]).  Then per token:
    s_e = x . v[e] + c[e],  logits = x @ Wg
    moe_sum = max(softmax(logits)) * s_argmax(logits)
    out = log_softmax over tokens (per batch row) of moe_sum.

Distribution over 8 cores, two launches (measured: a single ncfw collective
costs ~65us of barrier/trigger latency on this runtime — far more than a
second launch's fixed ~17us, so the 16KB cross-core combine happens on the
host between launches; the host does only that partial sum, all real math
stays on device):
  launch A (expert-parallel over H): core c reduces W2[:, 128c:128c+128, :]
    and computes partial v from the matching W1 columns (f32r stream after a
    rounding pass) -> outputs [v0 | v1 | c0 c1] partials (16KB); host sums.
  launch B (token-parallel): core c owns batch row c%4 (512 tokens): logits
    stream in fp32 (exact — argmax ties must match the reference), s stream
    in f32r, gate/select per token after a PE transpose, row log_softmax via
    PE transposes (no cross-partition DMA).  Host takes rows from cores 0..3.

Scheduling: stationary matmul operands are tiny (M<=4) so LDWEIGHTS is
negligible; fp32 streams at 4 cycles/row, f32r at 1.  Big loads alternate the
two HWDGE rings (SP via nc.sync, ACT via nc.scalar) for concurrency.  The d
axis is decomposed as d = p*16 + n so the flat v vector loads into [128,16]
tiles with contiguous per-partition runs.
"""

import sys

import numpy as np

for _p in ("/opt/trn_rl_repo",):
    if _p not in sys.path:
        sys.path.append(_p)

import concourse.bass as bass  # noqa: E402
import concourse.mybir as mybir  # noqa: E402
import concourse.tile as tile  # noqa: E402
from concourse import bacc, bass_utils  # noqa: E402
from concourse.masks import make_identity  # noqa: E402

# Problem shape (hardcoded per spec).
B, T, D, H, E = 4, 512, 2048, 1024, 2
P = 128
NCORES = 8
TB = T  # tokens per core = one batch row
NB = D // P  # 16 d-blocks
HC = H // NCORES  # 128 h-chunk per expert per core
NG = TB // P  # 4 token groups per core
DC = D // NCORES  # 256 b2 columns per core
VK = 4  # v computed in VK chunks of D/VK columns
F32 = mybir.dt.float32
F32R = mybir.dt.float32r
AX = mybir.AxisListType
AF = mybir.ActivationFunctionType
ALU = mybir.AluOpType

VPART = 2 * D + 2  # launch A output: v0 | v1 | c0 c1


def emit_phase_a(nc, tc, io):
    """w2sum + partial v for this core's H-chunk -> vpart [1, 2D+2]."""
    w1t, w2r, b1c, b2c, vout = io["w1t"], io["w2r"], io["b1c"], io["b2c"], io["vout"]
    with (
        tc.tile_pool(name="main", bufs=1) as pool,
        tc.tile_pool(name="psum", bufs=1, space="PSUM") as psum,
    ):
        # DMA plan: tiny contiguous bias rows FIRST on the sync ring (so no
        # DVE op ever head-of-line blocks on them), then W2 halves (they gate
        # the reduce), then W1 split over all three queues.  W1 goes straight
        # into an f32r tile (w1t is declared float32r) — no cast pass.
        HD = D // 2
        b1_sb = pool.tile([1, E * HC], F32)
        nc.sync.dma_start(b1_sb[:], b1c)
        b2_sb = pool.tile([1, E * DC], F32)
        nc.sync.dma_start(b2_sb[:], b2c)
        w2_sb = pool.tile([P, E, D], F32)
        w1r = pool.tile([P, E, D], F32R)
        for h in range(2):
            nc.sync.dma_start(w2_sb[:, 0, h * HD : (h + 1) * HD], w2r[0, :, h * HD : (h + 1) * HD])
            nc.scalar.dma_start(w2_sb[:, 1, h * HD : (h + 1) * HD], w2r[1, :, h * HD : (h + 1) * HD])
        nc.sync.dma_start(w1r[:, 0, 0:HD], w1t[0, :, 0:HD])
        nc.scalar.dma_start(w1r[:, 1, 0:HD], w1t[1, :, 0:HD])
        nc.gpsimd.dma_start(w1r[:, 0, HD:D], w1t[0, :, HD:D])
        nc.gpsimd.dma_start(w1r[:, 1, HD:D], w1t[1, :, HD:D])

        # b1 row -> partition-major [128, E] via PE transpose (identity [1,1])
        one1 = pool.tile([1, 1], F32)
        nc.gpsimd.memset(one1[:], 1.0)
        b1t_ps = psum.tile([P, E], F32)
        for e in range(E):
            nc.tensor.transpose(
                b1t_ps[:, e : e + 1], b1_sb[0:1, e * HC : (e + 1) * HC], one1[:]
            )
        b1p = pool.tile([P, E], F32)
        nc.vector.tensor_copy(b1p[:], b1t_ps[:])

        w2h = pool.tile([P, 2 * E], F32)
        w2s = pool.tile([P, E], F32)
        for e in range(E):
            for h in range(2):
                nc.vector.reduce_sum(
                    w2h[:, 2 * e + h : 2 * e + h + 1],
                    w2_sb[:, e, h * HD : (h + 1) * HD],
                    axis=AX.X,
                )
            nc.vector.tensor_add(
                w2s[:, e : e + 1], w2h[:, 2 * e : 2 * e + 1], w2h[:, 2 * e + 1 : 2 * e + 2]
            )
        w2s_r = pool.tile([P, E], F32R)
        nc.vector.tensor_copy(w2s_r[:], w2s[:])
        b2s = pool.tile([1, E], F32)
        for e in range(E):
            nc.vector.reduce_sum(
                b2s[0:1, e : e + 1], b2_sb[0:1, e * DC : (e + 1) * DC], axis=AX.X
            )

        pay = pool.tile([1, VPART], F32)
        b1dot = psum.tile([1, E], F32)
        DK = D // VK
        for e in range(E):
            for k in range(VK):
                vch = psum.tile([1, DK], F32, name="vch", tag="vch", bufs=2)
                nc.tensor.matmul(
                    vch[:],
                    w2s_r[:, e : e + 1],
                    w1r[:, e, k * DK : (k + 1) * DK],
                    start=True,
                    stop=True,
                )
                dst = pay[0:1, e * D + k * DK : e * D + (k + 1) * DK]
                if k % 2 == 0:
                    nc.vector.tensor_copy(dst, vch[:])
                else:
                    nc.scalar.copy(dst, vch[:])
            nc.tensor.matmul(
                b1dot[0:1, e : e + 1],
                w2s[:, e : e + 1],
                b1p[:, e : e + 1],
                start=True,
                stop=True,
            )
            nc.vector.tensor_add(
                pay[0:1, 2 * D + e : 2 * D + e + 1],
                b1dot[0:1, e : e + 1],
                b2s[0:1, e : e + 1],
            )
        nc.sync.dma_start(vout[:], pay[:])


def emit_phase_b(nc, tc, io):
    """logits (fp32) + s (f32r) streams, gate/select, row log_softmax."""
    xt, wgt, vin, out = io["xt"], io["wgt"], io["vin"], io["out"]
    rings = [nc.sync, nc.scalar]
    with (
        tc.tile_pool(name="main", bufs=1) as pool,
        tc.tile_pool(name="psum", bufs=1, space="PSUM") as psum,
    ):
        # descriptor-heavy small loads on the SWDGE queue (they'd clog the HW
        # rings: partition-major tiles emit one small packet per partition);
        # the HW rings carry pure x, graduated so block 0 lands ASAP
        wgt_sb = pool.tile([P, NB * E], F32)
        nc.gpsimd.dma_start(wgt_sb[:], wgt)
        wg3 = wgt_sb.rearrange("p (n e) -> p n e", e=E)
        vsb = pool.tile([P, E, NB], F32)
        for e in range(E):
            nc.gpsimd.dma_start(
                vsb[:, e, :],
                vin[0:1, e * D : (e + 1) * D].rearrange("x (p n) -> p (x n)", p=P),
            )
        csum = pool.tile([1, E], F32)
        nc.gpsimd.dma_start(csum[:], vin[0:1, 2 * D : 2 * D + E])

        x_sb = pool.tile([P, NB, TB], F32)
        xv = xt.rearrange("(p n) t -> p n t", p=P)
        qs = [nc.sync, nc.scalar, nc.gpsimd]
        chunks = [
            (0, 0, 1), (1, 1, 2),
            (0, 2, 4), (1, 4, 6),
            (0, 6, 9), (1, 9, 12),
            (2, 12, 16),
        ]
        for q, lo, hi in chunks:
            qs[q].dma_start(x_sb[:, lo:hi, :], xv[:, lo:hi, :])

        # preload ACT tables (Exp, Ln) off the critical path; keep ALL copy
        # work off the scalar engine so these tables are never evicted
        warm = pool.tile([1, 2], F32)
        nc.gpsimd.memset(warm[:], 1.0)
        wz = pool.tile([1, 2], F32)
        nc.scalar.activation(wz[:], warm[:], AF.Exp)
        nc.scalar.activation(wz[:], warm[:], AF.Ln)

        # small f32r prep FIRST (DVE is FIFO — these must not queue behind the
        # 4MB of x casts), then the x cast chain trailing the DMA chunks
        vsb_r = pool.tile([P, E, NB], F32R)
        nc.vector.tensor_copy(vsb_r[:], vsb[:])
        csum_b = pool.tile([P, E], F32)
        nc.gpsimd.partition_broadcast(csum_b[:], csum[0:1, :])
        x_r = pool.tile([P, NB, TB], F32R)
        for _, lo, hi in chunks:
            nc.vector.tensor_copy(x_r[:, lo:hi, :], x_sb[:, lo:hi, :])

        # logits stream (fp32 exact) fully first — the s stream's f32r inputs
        # (DVE casts) are ready long before the fp32 stream retires, and
        # keeping the streams separate avoids any PE head-of-line stall
        lg_ps = psum.tile([E, TB], F32)
        for n in range(NB):
            nc.tensor.matmul(
                lg_ps[:], wg3[:, n, :], x_sb[:, n, :], start=(n == 0), stop=(n == NB - 1)
            )
        sg_ps = psum.tile([E, TB], F32)
        for n in range(NB):
            nc.tensor.matmul(
                sg_ps[:], vsb_r[:, :, n], x_r[:, n, :], start=(n == 0), stop=(n == NB - 1)
            )
        sbl = pool.tile([E, TB], F32)
        nc.vector.tensor_copy(sbl[:], lg_ps[:])
        ident = pool.tile([P, P], F32)
        make_identity(nc, ident[:])

        gates, masks = [], []
        for g in range(NG):
            tpl = psum.tile([P, E], F32, name=f"tpl_{g}", tag="tp", bufs=2)
            nc.tensor.transpose(tpl[:], sbl[0:E, g * P : (g + 1) * P], ident[0:E, 0:E])
            t2l = pool.tile([P, E], F32, name=f"t2l_{g}")
            nc.vector.tensor_copy(t2l[:], tpl[:])
            negm = pool.tile([P, 1], F32, name=f"negm_{g}")
            nc.vector.reduce_max(negm[:], t2l[:], axis=AX.X, negate=True)
            z = pool.tile([P, E], F32, name=f"z_{g}")
            den = pool.tile([P, 1], F32, name=f"den_{g}")
            nc.scalar.activation(z[:], t2l[:], AF.Exp, bias=negm[:], accum_out=den[:])
            rec = pool.tile([P, 1], F32, name=f"rec_{g}")
            nc.vector.reciprocal(rec[:], den[:])
            zmax = pool.tile([P, 1], F32, name=f"zmax_{g}")
            nc.vector.reduce_max(zmax[:], z[:], axis=AX.X)
            gate = pool.tile([P, 1], F32, name=f"gate_{g}")
            nc.vector.tensor_mul(gate[:], zmax[:], rec[:])
            mask = pool.tile([P, 1], F32, name=f"mask_{g}")
            nc.vector.tensor_tensor(mask[:], t2l[:, 0:1], t2l[:, 1:2], op=ALU.is_ge)
            gates.append(gate)
            masks.append(mask)

        sbs = pool.tile([E, TB], F32)
        nc.vector.tensor_copy(sbs[:], sg_ps[:])

        moe_sb = pool.tile([P, NG], F32)
        for g in range(NG):
            tps = psum.tile([P, E], F32, name=f"tps_{g}", tag="tp", bufs=2)
            nc.tensor.transpose(tps[:], sbs[0:E, g * P : (g + 1) * P], ident[0:E, 0:E])
            t2s = pool.tile([P, E], F32, name=f"t2s_{g}")
            nc.vector.tensor_add(t2s[:], tps[:], csum_b[:])
            sdiff = pool.tile([P, 1], F32, name=f"sdiff_{g}")
            nc.vector.tensor_sub(sdiff[:], t2s[:, 0:1], t2s[:, 1:2])
            ssel = pool.tile([P, 1], F32, name=f"ssel_{g}")
            nc.vector.tensor_mul(ssel[:], masks[g][:], sdiff[:])
            nc.vector.tensor_add(ssel[:], ssel[:], t2s[:, 1:2])
            nc.vector.tensor_mul(moe_sb[:, g : g + 1], gates[g][:], ssel[:])

        # row log_softmax over all 512 tokens, via PE transposes
        tp4 = psum.tile([NG, P], F32)
        nc.tensor.transpose(tp4[:], moe_sb[:], ident[:])
        sb4t = pool.tile([NG, P], F32)
        nc.vector.tensor_copy(sb4t[:], tp4[:])
        m4p = pool.tile([NG, 1], F32)
        nc.vector.reduce_max(m4p[:], sb4t[:], axis=AX.X)
        m1p = psum.tile([1, NG], F32, name="m1p", tag="t1", bufs=2)
        nc.tensor.transpose(m1p[:], m4p[:], ident[0:NG, 0:NG])
        negm2 = pool.tile([1, 1], F32)
        nc.vector.reduce_max(negm2[:], m1p[:], axis=AX.X, negate=True)
        negm4 = pool.tile([NG, 1], F32)
        nc.gpsimd.partition_broadcast(negm4[:], negm2[:])
        e4 = pool.tile([NG, P], F32)
        s4 = pool.tile([NG, 1], F32)
        nc.scalar.activation(e4[:], sb4t[:], AF.Exp, bias=negm4[:], accum_out=s4[:])
        s1p = psum.tile([1, NG], F32, name="s1p", tag="t1", bufs=2)
        nc.tensor.transpose(s1p[:], s4[:], ident[0:NG, 0:NG])
        ssum = pool.tile([1, 1], F32)
        nc.vector.reduce_sum(ssum[:], s1p[:], axis=AX.X)
        logs = pool.tile([1, 1], F32)
        nc.scalar.activation(logs[:], ssum[:], AF.Ln)
        shift = pool.tile([1, 1], F32)
        nc.vector.tensor_sub(shift[:], negm2[:], logs[:])
        shift4 = pool.tile([NG, 1], F32)
        nc.gpsimd.partition_broadcast(shift4[:], shift[:])
        res4 = pool.tile([NG, P], F32)
        nc.vector.tensor_scalar_add(res4[:], sb4t[:], shift4[:])
        nc.sync.dma_start(out.rearrange("x (g p) -> g (x p)", p=P), res4[:])


_CACHED = {}


def build_program(which):
    if which in _CACHED:
        return _CACHED[which]
    nc = bacc.Bacc(
        "TRN2",
        target_bir_lowering=False,
        debug=False,
        enable_asserts=False,
        num_devices=NCORES,
    )
    if which == "a":
        io = {
            "w1t": nc.dram_tensor("w1t", [E, HC, D], F32R, kind="ExternalInput").ap(),
            "w2r": nc.dram_tensor("w2r", [E, HC, D], F32, kind="ExternalInput").ap(),
            "b1c": nc.dram_tensor("b1c", [1, E * HC], F32, kind="ExternalInput").ap(),
            "b2c": nc.dram_tensor("b2c", [1, E * DC], F32, kind="ExternalInput").ap(),
            "vout": nc.dram_tensor("vout", [1, VPART], F32, kind="ExternalOutput").ap(),
        }
        emit = emit_phase_a
    else:
        io = {
            "xt": nc.dram_tensor("xt", [D, TB], F32, kind="ExternalInput").ap(),
            "wgt": nc.dram_tensor("wgt", [P, NB * E], F32, kind="ExternalInput").ap(),
            "vin": nc.dram_tensor("vin", [1, VPART], F32, kind="ExternalInput").ap(),
            "out": nc.dram_tensor("out", [1, TB], F32, kind="ExternalOutput").ap(),
        }
        emit = emit_phase_b
    with tile.TileContext(nc) as tc:
        emit(nc, tc, io)
    nc.compile()
    _CACHED[which] = nc
    return nc


def shard_inputs_a(Wg, W1, b1, W2, b2):
    W1 = np.asarray(W1, np.float32)
    b1 = np.asarray(b1, np.float32)
    W2 = np.asarray(W2, np.float32)
    b2 = np.asarray(b2, np.float32)
    in_maps = []
    for c in range(NCORES):
        hs, he = c * HC, (c + 1) * HC
        in_maps.append(
            {
                "w1t": np.ascontiguousarray(W1[:, :, hs:he].transpose(0, 2, 1)),
                "w2r": np.ascontiguousarray(W2[:, hs:he, :]),
                "b1c": np.ascontiguousarray(b1[:, hs:he].reshape(1, E * HC)),
                "b2c": np.ascontiguousarray(
                    b2[:, c * DC : (c + 1) * DC].reshape(1, E * DC)
                ),
            }
        )
    return in_maps


def shard_inputs_b(x, Wg, vpart_sum):
    x = np.asarray(x, np.float32).reshape(B * T, D)
    Wg = np.asarray(Wg, np.float32)
    # wgt[p, n*2+e] = Wg[p*16+n, e]  (d = p*16 + n decomposition)
    wgt = np.ascontiguousarray(Wg.reshape(P, NB * E))
    in_maps = []
    for c in range(NCORES):
        row = c % B
        in_maps.append(
            {
                "xt": np.ascontiguousarray(x[row * TB : (row + 1) * TB, :].T),
                "wgt": wgt,
                "vin": vpart_sum,
            }
        )
    return in_maps


def run_a(in_maps, **kwargs):
    return bass_utils.run_bass_kernel_spmd(
        build_program("a"), in_maps, core_ids=list(range(NCORES)), **kwargs
    )


def run_b(in_maps, **kwargs):
    return bass_utils.run_bass_kernel_spmd(
        build_program("b"), in_maps, core_ids=list(range(NCORES)), **kwargs
    )


def kernel(x, Wg, W1, b1, W2, b2):
    res_a = run_a(shard_inputs_a(Wg, W1, b1, W2, b2))
    # cross-core combine: sum of the 8 per-core partials (the gather/reshard
    # step between the two launches; 16KB, no model math beyond the reduction)
    vpart = np.sum([res_a.results[c]["vout"] for c in range(NCORES)], axis=0)
    vpart = np.ascontiguousarray(vpart, np.float32)
    res_b = run_b(shard_inputs_b(x, Wg, vpart))
    return np.concatenate([res_b.results[b]["out"] for b in range(B)], axis=0)


# revision 34
# speedup vs baseline: 1.1108x; 1.0515x over previous
"""Trainium2 Bass kernel for nn_ExampleModel_1116691497724 (moe_routing).

Math: the reference returns log_softmax_T( sum_D(moe_out) ), and sum_D
collapses the expert FFN to a dot product:
    sum_d (h @ W2[e] + b2[e]) = h . w2sum[e] + sum(b2[e]),  w2sum[e] = W2[e] @ 1
    (x @ W1[e] + b1[e]) . w2sum[e] = x . v[e] + c[e]
with v[e] = W1[e] @ w2sum[e]  (a [D] vector) and scalar
c[e] = b1[e].w2sum[e] + sum(b2[e]).  Then per token:
    s_e = x . v[e] + c[e],  logits = x @ Wg
    moe_sum = max(softmax(logits)) * s_argmax(logits)
    out = log_softmax over tokens (per batch row) of moe_sum.

Distribution over 8 cores, two launches (measured: a single ncfw collective
costs ~65us of barrier/trigger latency on this runtime — far more than a
second launch's fixed ~17us, so the 16KB cross-core combine happens on the
host between launches; the host does only that partial sum, all real math
stays on device):
  launch A (expert-parallel over H): core c reduces W2[:, 128c:128c+128, :]
    and computes partial v from the matching W1 columns (f32r stream after a
    rounding pass) -> outputs [v0 | v1 | c0 c1] partials (16KB); host sums.
  launch B (token-parallel): core c owns batch row c%4 (512 tokens): logits
    stream in fp32 (exact — argmax ties must match the reference), s stream
    in f32r, gate/select per token after a PE transpose, row log_softmax via
    PE transposes (no cross-partition DMA).  Host takes rows from cores 0..3.

Scheduling: stationary matmul operands are tiny (M<=4) so LDWEIGHTS is
negligible; fp32 streams at 4 cycles/row, f32r at 1.  Big loads alternate the
two HWDGE rings (SP via nc.sync, ACT via nc.scalar) for concurrency.  The d
axis is decomposed as d = p*16 + n so the flat v vector loads into [128,16]
tiles with contiguous per-partition runs.
"""

import sys

import numpy as np

for _p in ("/opt/trn_rl_repo",):
    if _p not in sys.path:
        sys.path.append(_p)

import concourse.bass as bass  # noqa: E402
import concourse.mybir as mybir  # noqa: E402
import concourse.tile as tile  # noqa: E402
from concourse import bacc, bass_utils  # noqa: E402
from concourse.masks import make_identity  # noqa: E402

# Problem shape (hardcoded per spec).
B, T, D, H, E = 4, 512, 2048, 1024, 2
P = 128
NCORES = 8
TB = T  # tokens per core = one batch row
NB = D // P  # 16 d-blocks
HC = H // NCORES  # 128 h-chunk per expert per core
NG = TB // P  # 4 token groups per core
DC = D // NCORES  # 256 b2 columns per core
VK = 4  # v computed in VK chunks of D/VK columns
F32 = mybir.dt.float32
F32R = mybir.dt.float32r
AX = mybir.AxisListType
AF = mybir.ActivationFunctionType
ALU = mybir.AluOpType

VPART = 2 * D + 2  # launch A output: v0 | v1 | c0 c1


def emit_phase_a(nc, tc, io):
    """w2sum + partial v for this core's H-chunk -> vpart [1, 2D+2]."""
    w1t, w2r, b1c, b2c, vout = io["w1t"], io["w2r"], io["b1c"], io["b2c"], io["vout"]
    with (
        tc.tile_pool(name="main", bufs=1) as pool,
        tc.tile_pool(name="psum", bufs=1, space="PSUM") as psum,
    ):
        # DMA plan: tiny contiguous bias rows FIRST on the sync ring (so no
        # DVE op ever head-of-line blocks on them), then W2 halves (they gate
        # the reduce), then W1 split over all three queues.  W1 goes straight
        # into an f32r tile (w1t is declared float32r) — no cast pass.
        HD = D // 2
        b1_sb = pool.tile([1, E * HC], F32)
        nc.sync.dma_start(b1_sb[:], b1c)
        b2_sb = pool.tile([1, E * DC], F32)
        nc.sync.dma_start(b2_sb[:], b2c)
        w2_sb = pool.tile([P, E, D], F32)
        w1r = pool.tile([P, E, D], F32R)
        for h in range(2):
            nc.sync.dma_start(w2_sb[:, 0, h * HD : (h + 1) * HD], w2r[0, :, h * HD : (h + 1) * HD])
            nc.scalar.dma_start(w2_sb[:, 1, h * HD : (h + 1) * HD], w2r[1, :, h * HD : (h + 1) * HD])
        for h in range(2):
            nc.sync.dma_start(w1r[:, 0, h * HD : (h + 1) * HD], w1t[0, :, h * HD : (h + 1) * HD])
            nc.scalar.dma_start(w1r[:, 1, h * HD : (h + 1) * HD], w1t[1, :, h * HD : (h + 1) * HD])

        # b1 row -> partition-major [128, E] via PE transpose (identity [1,1])
        one1 = pool.tile([1, 1], F32)
        nc.gpsimd.memset(one1[:], 1.0)
        b1t_ps = psum.tile([P, E], F32)
        for e in range(E):
            nc.tensor.transpose(
                b1t_ps[:, e : e + 1], b1_sb[0:1, e * HC : (e + 1) * HC], one1[:]
            )
        b1p = pool.tile([P, E], F32)
        nc.vector.tensor_copy(b1p[:], b1t_ps[:])

        w2h = pool.tile([P, 2 * E], F32)
        w2s = pool.tile([P, E], F32)
        for e in range(E):
            for h in range(2):
                nc.vector.reduce_sum(
                    w2h[:, 2 * e + h : 2 * e + h + 1],
                    w2_sb[:, e, h * HD : (h + 1) * HD],
                    axis=AX.X,
                )
            nc.vector.tensor_add(
                w2s[:, e : e + 1], w2h[:, 2 * e : 2 * e + 1], w2h[:, 2 * e + 1 : 2 * e + 2]
            )
        w2s_r = pool.tile([P, E], F32R)
        nc.vector.tensor_copy(w2s_r[:], w2s[:])
        b2s = pool.tile([1, E], F32)
        for e in range(E):
            nc.vector.reduce_sum(
                b2s[0:1, e : e + 1], b2_sb[0:1, e * DC : (e + 1) * DC], axis=AX.X
            )

        pay = pool.tile([1, VPART], F32)
        b1dot = psum.tile([1, E], F32)
        DK = D // VK
        for e in range(E):
            for k in range(VK):
                vch = psum.tile([1, DK], F32, name="vch", tag="vch", bufs=2)
                nc.tensor.matmul(
                    vch[:],
                    w2s_r[:, e : e + 1],
                    w1r[:, e, k * DK : (k + 1) * DK],
                    start=True,
                    stop=True,
                )
                dst = pay[0:1, e * D + k * DK : e * D + (k + 1) * DK]
                if k % 2 == 0:
                    nc.vector.tensor_copy(dst, vch[:])
                else:
                    nc.scalar.copy(dst, vch[:])
            nc.tensor.matmul(
                b1dot[0:1, e : e + 1],
                w2s[:, e : e + 1],
                b1p[:, e : e + 1],
                start=True,
                stop=True,
            )
            nc.vector.tensor_add(
                pay[0:1, 2 * D + e : 2 * D + e + 1],
                b1dot[0:1, e : e + 1],
                b2s[0:1, e : e + 1],
            )
        nc.sync.dma_start(vout[:], pay[:])


def emit_phase_b(nc, tc, io):
    """logits (fp32) + s (f32r) streams, gate/select, row log_softmax."""
    xt, wgt, vin, out = io["xt"], io["wgt"], io["vin"], io["out"]
    rings = [nc.sync, nc.scalar]
    with (
        tc.tile_pool(name="main", bufs=1) as pool,
        tc.tile_pool(name="psum", bufs=1, space="PSUM") as psum,
    ):
        # descriptor-heavy small loads on the SWDGE queue (they'd clog the HW
        # rings: partition-major tiles emit one small packet per partition);
        # the HW rings carry pure x, graduated so block 0 lands ASAP
        wgt_sb = pool.tile([P, NB * E], F32)
        nc.gpsimd.dma_start(wgt_sb[:], wgt)
        wg3 = wgt_sb.rearrange("p (n e) -> p n e", e=E)
        vsb = pool.tile([P, E, NB], F32)
        for e in range(E):
            nc.gpsimd.dma_start(
                vsb[:, e, :],
                vin[0:1, e * D : (e + 1) * D].rearrange("x (p n) -> p (x n)", p=P),
            )
        csum = pool.tile([1, E], F32)
        nc.gpsimd.dma_start(csum[:], vin[0:1, 2 * D : 2 * D + E])

        x_sb = pool.tile([P, NB, TB], F32)
        xv = xt.rearrange("(p n) t -> p n t", p=P)
        qs = [nc.sync, nc.scalar]
        chunks = [
            (0, 0, 1), (1, 1, 2),
            (0, 2, 4), (1, 4, 6),
            (0, 6, 9), (1, 9, 12),
            (0, 12, 14), (1, 14, 16),
        ]
        for q, lo, hi in chunks:
            qs[q].dma_start(x_sb[:, lo:hi, :], xv[:, lo:hi, :])

        # preload ACT tables (Exp, Ln) off the critical path; keep ALL copy
        # work off the scalar engine so these tables are never evicted
        warm = pool.tile([1, 2], F32)
        nc.gpsimd.memset(warm[:], 1.0)
        wz = pool.tile([1, 2], F32)
        nc.scalar.activation(wz[:], warm[:], AF.Exp)
        nc.scalar.activation(wz[:], warm[:], AF.Ln)

        # small f32r prep FIRST (DVE is FIFO — these must not queue behind the
        # 4MB of x casts), then the x cast chain trailing the DMA chunks
        vsb_r = pool.tile([P, E, NB], F32R)
        nc.vector.tensor_copy(vsb_r[:], vsb[:])
        csum_b = pool.tile([P, E], F32)
        nc.gpsimd.partition_broadcast(csum_b[:], csum[0:1, :])
        x_r = pool.tile([P, NB, TB], F32R)
        for _, lo, hi in chunks:
            nc.vector.tensor_copy(x_r[:, lo:hi, :], x_sb[:, lo:hi, :])

        # logits stream (fp32 exact) fully first — the s stream's f32r inputs
        # (DVE casts) are ready long before the fp32 stream retires, and
        # keeping the streams separate avoids any PE head-of-line stall
        lg_ps = psum.tile([E, TB], F32)
        for n in range(NB):
            nc.tensor.matmul(
                lg_ps[:], wg3[:, n, :], x_sb[:, n, :], start=(n == 0), stop=(n == NB - 1)
            )
        sg_ps = psum.tile([E, TB], F32)
        for n in range(NB):
            nc.tensor.matmul(
                sg_ps[:], vsb_r[:, :, n], x_r[:, n, :], start=(n == 0), stop=(n == NB - 1)
            )
        sbl = pool.tile([E, TB], F32)
        nc.vector.tensor_copy(sbl[:], lg_ps[:])
        ident = pool.tile([P, P], F32)
        make_identity(nc, ident[:])

        gates, masks = [], []
        for g in range(NG):
            tpl = psum.tile([P, E], F32, name=f"tpl_{g}", tag="tp", bufs=2)
            nc.tensor.transpose(tpl[:], sbl[0:E, g * P : (g + 1) * P], ident[0:E, 0:E])
            t2l = pool.tile([P, E], F32, name=f"t2l_{g}")
            nc.vector.tensor_copy(t2l[:], tpl[:])
            negm = pool.tile([P, 1], F32, name=f"negm_{g}")
            nc.vector.reduce_max(negm[:], t2l[:], axis=AX.X, negate=True)
            z = pool.tile([P, E], F32, name=f"z_{g}")
            den = pool.tile([P, 1], F32, name=f"den_{g}")
            nc.scalar.activation(z[:], t2l[:], AF.Exp, bias=negm[:], accum_out=den[:])
            rec = pool.tile([P, 1], F32, name=f"rec_{g}")
            nc.vector.reciprocal(rec[:], den[:])
            zmax = pool.tile([P, 1], F32, name=f"zmax_{g}")
            nc.vector.reduce_max(zmax[:], z[:], axis=AX.X)
            gate = pool.tile([P, 1], F32, name=f"gate_{g}")
            nc.vector.tensor_mul(gate[:], zmax[:], rec[:])
            mask = pool.tile([P, 1], F32, name=f"mask_{g}")
            nc.vector.tensor_tensor(mask[:], t2l[:, 0:1], t2l[:, 1:2], op=ALU.is_ge)
            gates.append(gate)
            masks.append(mask)

        sbs = pool.tile([E, TB], F32)
        nc.vector.tensor_copy(sbs[:], sg_ps[:])

        moe_sb = pool.tile([P, NG], F32)
        for g in range(NG):
            tps = psum.tile([P, E], F32, name=f"tps_{g}", tag="tp", bufs=2)
            nc.tensor.transpose(tps[:], sbs[0:E, g * P : (g + 1) * P], ident[0:E, 0:E])
            t2s = pool.tile([P, E], F32, name=f"t2s_{g}")
            nc.vector.tensor_add(t2s[:], tps[:], csum_b[:])
            sdiff = pool.tile([P, 1], F32, name=f"sdiff_{g}")
            nc.vector.tensor_sub(sdiff[:], t2s[:, 0:1], t2s[:, 1:2])
            ssel = pool.tile([P, 1], F32, name=f"ssel_{g}")
            nc.vector.tensor_mul(ssel[:], masks[g][:], sdiff[:])
            nc.vector.tensor_add(ssel[:], ssel[:], t2s[:, 1:2])
            nc.vector.tensor_mul(moe_sb[:, g : g + 1], gates[g][:], ssel[:])

        # row log_softmax over all 512 tokens, via PE transposes
        tp4 = psum.tile([NG, P], F32)
        nc.tensor.transpose(tp4[:], moe_sb[:], ident[:])
        sb4t = pool.tile([NG, P], F32)
        nc.vector.tensor_copy(sb4t[:], tp4[:])
        m4p = pool.tile([NG, 1], F32)
        nc.vector.reduce_max(m4p[:], sb4t[:], axis=AX.X)
        m1p = psum.tile([1, NG], F32, name="m1p", tag="t1", bufs=2)
        nc.tensor.transpose(m1p[:], m4p[:], ident[0:NG, 0:NG])
        negm2 = pool.tile([1, 1], F32)
        nc.vector.reduce_max(negm2[:], m1p[:], axis=AX.X, negate=True)
        negm4 = pool.tile([NG, 1], F32)
        nc.gpsimd.partition_broadcast(negm4[:], negm2[:])
        e4 = pool.tile([NG, P], F32)
        s4 = pool.tile([NG, 1], F32)
        nc.scalar.activation(e4[:], sb4t[:], AF.Exp, bias=negm4[:], accum_out=s4[:])
        s1p = psum.tile([1, NG], F32, name="s1p", tag="t1", bufs=2)
        nc.tensor.transpose(s1p[:], s4[:], ident[0:NG, 0:NG])
        ssum = pool.tile([1, 1], F32)
        nc.vector.reduce_sum(ssum[:], s1p[:], axis=AX.X)
        logs = pool.tile([1, 1], F32)
        nc.scalar.activation(logs[:], ssum[:], AF.Ln)
        shift = pool.tile([1, 1], F32)
        nc.vector.tensor_sub(shift[:], negm2[:], logs[:])
        shift4 = pool.tile([NG, 1], F32)
        nc.gpsimd.partition_broadcast(shift4[:], shift[:])
        res4 = pool.tile([NG, P], F32)
        nc.vector.tensor_scalar_add(res4[:], sb4t[:], shift4[:])
        nc.sync.dma_start(out.rearrange("x (g p) -> g (x p)", p=P), res4[:])


_CACHED = {}


def build_program(which):
    if which in _CACHED:
        return _CACHED[which]
    nc = bacc.Bacc(
        "TRN2",
        target_bir_lowering=False,
        debug=False,
        enable_asserts=False,
        num_devices=NCORES,
    )
    if which == "a":
        io = {
            "w1t": nc.dram_tensor("w1t", [E, HC, D], F32R, kind="ExternalInput").ap(),
            "w2r": nc.dram_tensor("w2r", [E, HC, D], F32, kind="ExternalInput").ap(),
            "b1c": nc.dram_tensor("b1c", [1, E * HC], F32, kind="ExternalInput").ap(),
            "b2c": nc.dram_tensor("b2c", [1, E * DC], F32, kind="ExternalInput").ap(),
            "vout": nc.dram_tensor("vout", [1, VPART], F32, kind="ExternalOutput").ap(),
        }
        emit = emit_phase_a
    else:
        io = {
            "xt": nc.dram_tensor("xt", [D, TB], F32, kind="ExternalInput").ap(),
            "wgt": nc.dram_tensor("wgt", [P, NB * E], F32, kind="ExternalInput").ap(),
            "vin": nc.dram_tensor("vin", [1, VPART], F32, kind="ExternalInput").ap(),
            "out": nc.dram_tensor("out", [1, TB], F32, kind="ExternalOutput").ap(),
        }
        emit = emit_phase_b
    with tile.TileContext(nc) as tc:
        emit(nc, tc, io)
    nc.compile()
    _CACHED[which] = nc
    return nc


def shard_inputs_a(Wg, W1, b1, W2, b2):
    W1 = np.asarray(W1, np.float32)
    b1 = np.asarray(b1, np.float32)
    W2 = np.asarray(W2, np.float32)
    b2 = np.asarray(b2, np.float32)
    in_maps = []
    for c in range(NCORES):
        hs, he = c * HC, (c + 1) * HC
        in_maps.append(
            {
                "w1t": np.ascontiguousarray(W1[:, :, hs:he].transpose(0, 2, 1)),
                "w2r": np.ascontiguousarray(W2[:, hs:he, :]),
                "b1c": np.ascontiguousarray(b1[:, hs:he].reshape(1, E * HC)),
                "b2c": np.ascontiguousarray(
                    b2[:, c * DC : (c + 1) * DC].reshape(1, E * DC)
                ),
            }
        )
    return in_maps


def shard_inputs_b(x, Wg, vpart_sum):
    x = np.asarray(x, np.float32).reshape(B * T, D)
    Wg = np.asarray(Wg, np.float32)
    # wgt[p, n*2+e] = Wg[p*16+n, e]  (d = p*16 + n decomposition)
    wgt = np.ascontiguousarray(Wg.reshape(P, NB * E))
    in_maps = []
    for c in range(NCORES):
        row = c % B
        in_maps.append(
            {
                "xt": np.ascontiguousarray(x[row * TB : (row + 1) * TB, :].T),
                "wgt": wgt,
                "vin": vpart_sum,
            }
        )
    return in_maps


def run_a(in_maps, **kwargs):
    return bass_utils.run_bass_kernel_spmd(
        build_program("a"), in_maps, core_ids=list(range(NCORES)), **kwargs
    )


def run_b(in_maps, **kwargs):
    return bass_utils.run_bass_kernel_spmd(
        build_program("b"), in_maps, core_ids=list(range(NCORES)), **kwargs
    )


def kernel(x, Wg, W1, b1, W2, b2):
    res_a = run_a(shard_inputs_a(Wg, W1, b1, W2, b2))
    # cross-core combine: sum of the 8 per-core partials (the gather/reshard
    # step between the two launches; 16KB, no model math beyond the reduction)
    vpart = np.sum([res_a.results[c]["vout"] for c in range(NCORES)], axis=0)
    vpart = np.ascontiguousarray(vpart, np.float32)
    res_b = run_b(shard_inputs_b(x, Wg, vpart))
    return np.concatenate([res_b.results[b]["out"] for b in range(B)], axis=0)


# revision 39
# speedup vs baseline: 1.1347x; 1.0215x over previous
"""Trainium2 Bass kernel for nn_ExampleModel_1116691497724 (moe_routing).

Math: the reference returns log_softmax_T( sum_D(moe_out) ), and sum_D
collapses the expert FFN to a dot product:
    sum_d (h @ W2[e] + b2[e]) = h . w2sum[e] + sum(b2[e]),  w2sum[e] = W2[e] @ 1
    (x @ W1[e] + b1[e]) . w2sum[e] = x . v[e] + c[e]
with v[e] = W1[e] @ w2sum[e]  (a [D] vector) and scalar
c[e] = b1[e].w2sum[e] + sum(b2[e]).  Then per token:
    s_e = x . v[e] + c[e],  logits = x @ Wg
    moe_sum = max(softmax(logits)) * s_argmax(logits)
    out = log_softmax over tokens (per batch row) of moe_sum.

Distribution over 8 cores, two launches (measured: a single ncfw collective
costs ~65us of barrier/trigger latency on this runtime — far more than a
second launch's fixed ~17us, so the 16KB cross-core combine happens on the
host between launches; the host does only that partial sum, all real math
stays on device):
  launch A (expert-parallel over H): core c reduces W2[:, 128c:128c+128, :]
    and computes partial v from the matching W1 columns (f32r stream after a
    rounding pass) -> outputs [v0 | v1 | c0 c1] partials (16KB); host sums.
  launch B (token-parallel): core c owns batch row c%4 (512 tokens): logits
    stream in fp32 (exact — argmax ties must match the reference), s stream
    in f32r, gate/select per token after a PE transpose, row log_softmax via
    PE transposes (no cross-partition DMA).  Host takes rows from cores 0..3.

Scheduling: stationary matmul operands are tiny (M<=4) so LDWEIGHTS is
negligible; fp32 streams at 4 cycles/row, f32r at 1.  Big loads alternate the
two HWDGE rings (SP via nc.sync, ACT via nc.scalar) for concurrency.  The d
axis is decomposed as d = p*16 + n so the flat v vector loads into [128,16]
tiles with contiguous per-partition runs.
"""

import sys

import numpy as np

for _p in ("/opt/trn_rl_repo",):
    if _p not in sys.path:
        sys.path.append(_p)

import concourse.bass as bass  # noqa: E402
import concourse.mybir as mybir  # noqa: E402
import concourse.tile as tile  # noqa: E402
from concourse import bacc, bass_utils  # noqa: E402
from concourse.masks import make_identity  # noqa: E402

# Problem shape (hardcoded per spec).
B, T, D, H, E = 4, 512, 2048, 1024, 2
P = 128
NCORES = 8
TB = T  # tokens per core = one batch row
NB = D // P  # 16 d-blocks
HC = H // NCORES  # 128 h-chunk per expert per core
NG = TB // P  # 4 token groups per core
DC = D // NCORES  # 256 b2 columns per core
VK = 4  # v computed in VK chunks of D/VK columns
F32 = mybir.dt.float32
F32R = mybir.dt.float32r
AX = mybir.AxisListType
AF = mybir.ActivationFunctionType
ALU = mybir.AluOpType

VPART = 2 * D + 2  # launch A output: v0 | v1 | c0 c1
BF16 = mybir.dt.bfloat16
BF16_W = True  # load W1/W2 as bf16 (halves launch A DMA; v is smooth-path only)


def emit_phase_a(nc, tc, io):
    """w2sum + partial v for this core's H-chunk -> vpart [1, 2D+2]."""
    w1t, w2r, b1c, b2c, vout = io["w1t"], io["w2r"], io["b1c"], io["b2c"], io["vout"]
    with (
        tc.tile_pool(name="main", bufs=1) as pool,
        tc.tile_pool(name="psum", bufs=1, space="PSUM") as psum,
    ):
        # DMA plan: tiny contiguous bias rows FIRST on the sync ring (so no
        # DVE op ever head-of-line blocks on them), then W2 halves (they gate
        # the reduce), then W1 split over all three queues.  W1 goes straight
        # into an f32r tile (w1t is declared float32r) — no cast pass.
        HD = D // 2
        WDT = BF16 if BF16_W else F32
        VDT = BF16 if BF16_W else F32R
        b1_sb = pool.tile([1, E * HC], F32)
        nc.sync.dma_start(b1_sb[:], b1c)
        b2_sb = pool.tile([1, E * DC], F32)
        nc.sync.dma_start(b2_sb[:], b2c)
        w2_sb = pool.tile([P, E, D], WDT)
        w1r = pool.tile([P, E, D], VDT)
        for h in range(2):
            nc.sync.dma_start(w2_sb[:, 0, h * HD : (h + 1) * HD], w2r[0, :, h * HD : (h + 1) * HD])
            nc.scalar.dma_start(w2_sb[:, 1, h * HD : (h + 1) * HD], w2r[1, :, h * HD : (h + 1) * HD])
        for h in range(2):
            nc.sync.dma_start(w1r[:, 0, h * HD : (h + 1) * HD], w1t[0, :, h * HD : (h + 1) * HD])
            nc.scalar.dma_start(w1r[:, 1, h * HD : (h + 1) * HD], w1t[1, :, h * HD : (h + 1) * HD])

        # b1 row -> partition-major [128, E] via PE transpose (identity [1,1])
        one1 = pool.tile([1, 1], F32)
        nc.gpsimd.memset(one1[:], 1.0)
        b1t_ps = psum.tile([P, E], F32)
        for e in range(E):
            nc.tensor.transpose(
                b1t_ps[:, e : e + 1], b1_sb[0:1, e * HC : (e + 1) * HC], one1[:]
            )
        b1p = pool.tile([P, E], F32)
        nc.vector.tensor_copy(b1p[:], b1t_ps[:])

        w2h = pool.tile([P, 2 * E], F32)
        w2s = pool.tile([P, E], F32)
        for e in range(E):
            for h in range(2):
                nc.vector.reduce_sum(
                    w2h[:, 2 * e + h : 2 * e + h + 1],
                    w2_sb[:, e, h * HD : (h + 1) * HD],
                    axis=AX.X,
                )
            nc.vector.tensor_add(
                w2s[:, e : e + 1], w2h[:, 2 * e : 2 * e + 1], w2h[:, 2 * e + 1 : 2 * e + 2]
            )
        w2s_r = pool.tile([P, E], VDT)
        nc.vector.tensor_copy(w2s_r[:], w2s[:])
        b2s = pool.tile([1, E], F32)
        for e in range(E):
            nc.vector.reduce_sum(
                b2s[0:1, e : e + 1], b2_sb[0:1, e * DC : (e + 1) * DC], axis=AX.X
            )

        pay = pool.tile([1, VPART], F32)
        b1dot = psum.tile([1, E], F32)
        DK = D // VK
        for e in range(E):
            for k in range(VK):
                vch = psum.tile([1, DK], F32, name="vch", tag="vch", bufs=2)
                nc.tensor.matmul(
                    vch[:],
                    w2s_r[:, e : e + 1],
                    w1r[:, e, k * DK : (k + 1) * DK],
                    start=True,
                    stop=True,
                )
                dst = pay[0:1, e * D + k * DK : e * D + (k + 1) * DK]
                if k % 2 == 0:
                    nc.vector.tensor_copy(dst, vch[:])
                else:
                    nc.scalar.copy(dst, vch[:])
            nc.tensor.matmul(
                b1dot[0:1, e : e + 1],
                w2s[:, e : e + 1],
                b1p[:, e : e + 1],
                start=True,
                stop=True,
            )
            nc.vector.tensor_add(
                pay[0:1, 2 * D + e : 2 * D + e + 1],
                b1dot[0:1, e : e + 1],
                b2s[0:1, e : e + 1],
            )
        nc.sync.dma_start(vout[:], pay[:])


def emit_phase_b(nc, tc, io):
    """logits (fp32) + s (f32r) streams, gate/select, row log_softmax."""
    xt, wgt, vin, out = io["xt"], io["wgt"], io["vin"], io["out"]
    rings = [nc.sync, nc.scalar]
    with (
        tc.tile_pool(name="main", bufs=1) as pool,
        tc.tile_pool(name="psum", bufs=1, space="PSUM") as psum,
    ):
        # descriptor-heavy small loads on the SWDGE queue (they'd clog the HW
        # rings: partition-major tiles emit one small packet per partition);
        # the HW rings carry pure x, graduated so block 0 lands ASAP
        wgt_sb = pool.tile([P, NB * E], F32)
        nc.gpsimd.dma_start(wgt_sb[:], wgt)
        wg3 = wgt_sb.rearrange("p (n e) -> p n e", e=E)
        vsb = pool.tile([P, E, NB], F32)
        for e in range(E):
            nc.gpsimd.dma_start(
                vsb[:, e, :],
                vin[0:1, e * D : (e + 1) * D].rearrange("x (p n) -> p (x n)", p=P),
            )
        csum = pool.tile([1, E], F32)
        nc.gpsimd.dma_start(csum[:], vin[0:1, 2 * D : 2 * D + E])

        x_sb = pool.tile([P, NB, TB], F32)
        xv = xt.rearrange("(p n) t -> p n t", p=P)
        qs = [nc.sync, nc.scalar]
        chunks = [
            (0, 0, 1), (1, 1, 2),
            (0, 2, 4), (1, 4, 6),
            (0, 6, 9), (1, 9, 12),
            (0, 12, 14), (1, 14, 16),
        ]
        for q, lo, hi in chunks:
            qs[q].dma_start(x_sb[:, lo:hi, :], xv[:, lo:hi, :])

        # preload ACT tables (Exp, Ln) off the critical path; keep ALL copy
        # work off the scalar engine so these tables are never evicted
        warm = pool.tile([1, 2], F32)
        nc.gpsimd.memset(warm[:], 1.0)
        wz = pool.tile([1, 2], F32)
        nc.scalar.activation(wz[:], warm[:], AF.Exp)
        nc.scalar.activation(wz[:], warm[:], AF.Ln)

        # small f32r prep FIRST (DVE is FIFO — these must not queue behind the
        # 4MB of x casts), then the x cast chain trailing the DMA chunks
        vsb_r = pool.tile([P, E, NB], F32R)
        nc.vector.tensor_copy(vsb_r[:], vsb[:])
        csum_b = pool.tile([P, E], F32)
        nc.gpsimd.partition_broadcast(csum_b[:], csum[0:1, :])
        x_r = pool.tile([P, NB, TB], F32R)
        for _, lo, hi in chunks:
            nc.vector.tensor_copy(x_r[:, lo:hi, :], x_sb[:, lo:hi, :])

        # logits stream (fp32 exact) fully first — the s stream's f32r inputs
        # (DVE casts) are ready long before the fp32 stream retires, and
        # keeping the streams separate avoids any PE head-of-line stall
        lg_ps = psum.tile([E, TB], F32)
        for n in range(NB):
            nc.tensor.matmul(
                lg_ps[:], wg3[:, n, :], x_sb[:, n, :], start=(n == 0), stop=(n == NB - 1)
            )
        sg_ps = psum.tile([E, TB], F32)
        for n in range(NB):
            nc.tensor.matmul(
                sg_ps[:], vsb_r[:, :, n], x_r[:, n, :], start=(n == 0), stop=(n == NB - 1)
            )
        sbl = pool.tile([E, TB], F32)
        nc.vector.tensor_copy(sbl[:], lg_ps[:])
        ident = pool.tile([P, P], F32)
        make_identity(nc, ident[:])

        gates, masks = [], []
        for g in range(NG):
            tpl = psum.tile([P, E], F32, name=f"tpl_{g}", tag="tp", bufs=2)
            nc.tensor.transpose(tpl[:], sbl[0:E, g * P : (g + 1) * P], ident[0:E, 0:E])
            t2l = pool.tile([P, E], F32, name=f"t2l_{g}")
            nc.vector.tensor_copy(t2l[:], tpl[:])
            negm = pool.tile([P, 1], F32, name=f"negm_{g}")
            nc.vector.reduce_max(negm[:], t2l[:], axis=AX.X, negate=True)
            z = pool.tile([P, E], F32, name=f"z_{g}")
            den = pool.tile([P, 1], F32, name=f"den_{g}")
            nc.scalar.activation(z[:], t2l[:], AF.Exp, bias=negm[:], accum_out=den[:])
            rec = pool.tile([P, 1], F32, name=f"rec_{g}")
            nc.vector.reciprocal(rec[:], den[:])
            zmax = pool.tile([P, 1], F32, name=f"zmax_{g}")
            nc.vector.reduce_max(zmax[:], z[:], axis=AX.X)
            gate = pool.tile([P, 1], F32, name=f"gate_{g}")
            nc.vector.tensor_mul(gate[:], zmax[:], rec[:])
            mask = pool.tile([P, 1], F32, name=f"mask_{g}")
            nc.vector.tensor_tensor(mask[:], t2l[:, 0:1], t2l[:, 1:2], op=ALU.is_ge)
            gates.append(gate)
            masks.append(mask)

        sbs = pool.tile([E, TB], F32)
        nc.vector.tensor_copy(sbs[:], sg_ps[:])

        moe_sb = pool.tile([P, NG], F32)
        for g in range(NG):
            tps = psum.tile([P, E], F32, name=f"tps_{g}", tag="tp", bufs=2)
            nc.tensor.transpose(tps[:], sbs[0:E, g * P : (g + 1) * P], ident[0:E, 0:E])
            t2s = pool.tile([P, E], F32, name=f"t2s_{g}")
            nc.vector.tensor_add(t2s[:], tps[:], csum_b[:])
            sdiff = pool.tile([P, 1], F32, name=f"sdiff_{g}")
            nc.vector.tensor_sub(sdiff[:], t2s[:, 0:1], t2s[:, 1:2])
            ssel = pool.tile([P, 1], F32, name=f"ssel_{g}")
            nc.vector.tensor_mul(ssel[:], masks[g][:], sdiff[:])
            nc.vector.tensor_add(ssel[:], ssel[:], t2s[:, 1:2])
            nc.vector.tensor_mul(moe_sb[:, g : g + 1], gates[g][:], ssel[:])

        # row log_softmax over all 512 tokens, via PE transposes
        tp4 = psum.tile([NG, P], F32)
        nc.tensor.transpose(tp4[:], moe_sb[:], ident[:])
        sb4t = pool.tile([NG, P], F32)
        nc.vector.tensor_copy(sb4t[:], tp4[:])
        m4p = pool.tile([NG, 1], F32)
        nc.vector.reduce_max(m4p[:], sb4t[:], axis=AX.X)
        m1p = psum.tile([1, NG], F32, name="m1p", tag="t1", bufs=2)
        nc.tensor.transpose(m1p[:], m4p[:], ident[0:NG, 0:NG])
        negm2 = pool.tile([1, 1], F32)
        nc.vector.reduce_max(negm2[:], m1p[:], axis=AX.X, negate=True)
        negm4 = pool.tile([NG, 1], F32)
        nc.gpsimd.partition_broadcast(negm4[:], negm2[:])
        e4 = pool.tile([NG, P], F32)
        s4 = pool.tile([NG, 1], F32)
        nc.scalar.activation(e4[:], sb4t[:], AF.Exp, bias=negm4[:], accum_out=s4[:])
        s1p = psum.tile([1, NG], F32, name="s1p", tag="t1", bufs=2)
        nc.tensor.transpose(s1p[:], s4[:], ident[0:NG, 0:NG])
        ssum = pool.tile([1, 1], F32)
        nc.vector.reduce_sum(ssum[:], s1p[:], axis=AX.X)
        logs = pool.tile([1, 1], F32)
        nc.scalar.activation(logs[:], ssum[:], AF.Ln)
        shift = pool.tile([1, 1], F32)
        nc.vector.tensor_sub(shift[:], negm2[:], logs[:])
        shift4 = pool.tile([NG, 1], F32)
        nc.gpsimd.partition_broadcast(shift4[:], shift[:])
        res4 = pool.tile([NG, P], F32)
        nc.vector.tensor_scalar_add(res4[:], sb4t[:], shift4[:])
        nc.sync.dma_start(out.rearrange("x (g p) -> g (x p)", p=P), res4[:])


_CACHED = {}


def build_program(which):
    if which in _CACHED:
        return _CACHED[which]
    nc = bacc.Bacc(
        "TRN2",
        target_bir_lowering=False,
        debug=False,
        enable_asserts=False,
        num_devices=NCORES,
    )
    if which == "a":
        io = {
            "w1t": nc.dram_tensor(
                "w1t", [E, HC, D], BF16 if BF16_W else F32R, kind="ExternalInput"
            ).ap(),
            "w2r": nc.dram_tensor(
                "w2r", [E, HC, D], BF16 if BF16_W else F32, kind="ExternalInput"
            ).ap(),
            "b1c": nc.dram_tensor("b1c", [1, E * HC], F32, kind="ExternalInput").ap(),
            "b2c": nc.dram_tensor("b2c", [1, E * DC], F32, kind="ExternalInput").ap(),
            "vout": nc.dram_tensor("vout", [1, VPART], F32, kind="ExternalOutput").ap(),
        }
        emit = emit_phase_a
    else:
        io = {
            "xt": nc.dram_tensor("xt", [D, TB], F32, kind="ExternalInput").ap(),
            "wgt": nc.dram_tensor("wgt", [P, NB * E], F32, kind="ExternalInput").ap(),
            "vin": nc.dram_tensor("vin", [1, VPART], F32, kind="ExternalInput").ap(),
            "out": nc.dram_tensor("out", [1, TB], F32, kind="ExternalOutput").ap(),
        }
        emit = emit_phase_b
    with tile.TileContext(nc) as tc:
        emit(nc, tc, io)
    nc.compile()
    _CACHED[which] = nc
    return nc


def shard_inputs_a(Wg, W1, b1, W2, b2):
    import ml_dtypes

    wdt = ml_dtypes.bfloat16 if BF16_W else np.float32
    W1 = np.asarray(W1, np.float32)
    b1 = np.asarray(b1, np.float32)
    W2 = np.asarray(W2, np.float32)
    b2 = np.asarray(b2, np.float32)
    in_maps = []
    for c in range(NCORES):
        hs, he = c * HC, (c + 1) * HC
        in_maps.append(
            {
                "w1t": np.ascontiguousarray(W1[:, :, hs:he].transpose(0, 2, 1).astype(wdt)),
                "w2r": np.ascontiguousarray(W2[:, hs:he, :].astype(wdt)),
                "b1c": np.ascontiguousarray(b1[:, hs:he].reshape(1, E * HC)),
                "b2c": np.ascontiguousarray(
                    b2[:, c * DC : (c + 1) * DC].reshape(1, E * DC)
                ),
            }
        )
    return in_maps


def shard_inputs_b(x, Wg, vpart_sum):
    x = np.asarray(x, np.float32).reshape(B * T, D)
    Wg = np.asarray(Wg, np.float32)
    # wgt[p, n*2+e] = Wg[p*16+n, e]  (d = p*16 + n decomposition)
    wgt = np.ascontiguousarray(Wg.reshape(P, NB * E))
    in_maps = []
    for c in range(NCORES):
        row = c % B
        in_maps.append(
            {
                "xt": np.ascontiguousarray(x[row * TB : (row + 1) * TB, :].T),
                "wgt": wgt,
                "vin": vpart_sum,
            }
        )
    return in_maps


def run_a(in_maps, **kwargs):
    return bass_utils.run_bass_kernel_spmd(
        build_program("a"), in_maps, core_ids=list(range(NCORES)), **kwargs
    )


def run_b(in_maps, **kwargs):
    return bass_utils.run_bass_kernel_spmd(
        build_program("b"), in_maps, core_ids=list(range(NCORES)), **kwargs
    )


def kernel(x, Wg, W1, b1, W2, b2):
    res_a = run_a(shard_inputs_a(Wg, W1, b1, W2, b2))
    # cross-core combine: sum of the 8 per-core partials (the gather/reshard
    # step between the two launches; 16KB, no model math beyond the reduction)
    vpart = np.sum([res_a.results[c]["vout"] for c in range(NCORES)], axis=0)
    vpart = np.ascontiguousarray(vpart, np.float32)
    res_b = run_b(shard_inputs_b(x, Wg, vpart))
    return np.concatenate([res_b.results[b]["out"] for b in range(B)], axis=0)


# revision 40
# speedup vs baseline: 1.1837x; 1.0432x over previous
"""Trainium2 Bass kernel for nn_ExampleModel_1116691497724 (moe_routing).

Math: the reference returns log_softmax_T( sum_D(moe_out) ), and sum_D
collapses the expert FFN to a dot product:
    sum_d (h @ W2[e] + b2[e]) = h . w2sum[e] + sum(b2[e]),  w2sum[e] = W2[e] @ 1
    (x @ W1[e] + b1[e]) . w2sum[e] = x . v[e] + c[e]
with v[e] = W1[e] @ w2sum[e]  (a [D] vector) and scalar
c[e] = b1[e].w2sum[e] + sum(b2[e]).  Then per token:
    s_e = x . v[e] + c[e],  logits = x @ Wg
    moe_sum = max(softmax(logits)) * s_argmax(logits)
    out = log_softmax over tokens (per batch row) of moe_sum.

Distribution over 8 cores, two launches (measured: a single ncfw collective
costs ~65us of barrier/trigger latency on this runtime — far more than a
second launch's fixed ~17us, so the 16KB cross-core combine happens on the
host between launches; the host does only that partial sum, all real math
stays on device):
  launch A (expert-parallel over H): core c reduces W2[:, 128c:128c+128, :]
    and computes partial v from the matching W1 columns (f32r stream after a
    rounding pass) -> outputs [v0 | v1 | c0 c1] partials (16KB); host sums.
  launch B (token-parallel): core c owns batch row c%4 (512 tokens): logits
    stream in fp32 (exact — argmax ties must match the reference), s stream
    in f32r, gate/select per token after a PE transpose, row log_softmax via
    PE transposes (no cross-partition DMA).  Host takes rows from cores 0..3.

Scheduling: stationary matmul operands are tiny (M<=4) so LDWEIGHTS is
negligible; fp32 streams at 4 cycles/row, f32r at 1.  Big loads alternate the
two HWDGE rings (SP via nc.sync, ACT via nc.scalar) for concurrency.  The d
axis is decomposed as d = p*16 + n so the flat v vector loads into [128,16]
tiles with contiguous per-partition runs.
"""

import sys

import numpy as np

for _p in ("/opt/trn_rl_repo",):
    if _p not in sys.path:
        sys.path.append(_p)

import concourse.bass as bass  # noqa: E402
import concourse.mybir as mybir  # noqa: E402
import concourse.tile as tile  # noqa: E402
from concourse import bacc, bass_utils  # noqa: E402
from concourse.masks import make_identity  # noqa: E402

# Problem shape (hardcoded per spec).
B, T, D, H, E = 4, 512, 2048, 1024, 2
P = 128
NCORES = 8
TB = T  # tokens per core = one batch row
NB = D // P  # 16 d-blocks
HC = H // NCORES  # 128 h-chunk per expert per core
NG = TB // P  # 4 token groups per core
DC = D // NCORES  # 256 b2 columns per core
VK = 4  # v computed in VK chunks of D/VK columns
F32 = mybir.dt.float32
F32R = mybir.dt.float32r
AX = mybir.AxisListType
AF = mybir.ActivationFunctionType
ALU = mybir.AluOpType

VPART = 2 * D + 2  # launch A output: v0 | v1 | c0 c1
BF16 = mybir.dt.bfloat16
BF16_W = False  # bf16 W1/W2 saves only ~2us but costs 13x accuracy; keep f32


def emit_phase_a(nc, tc, io):
    """w2sum + partial v for this core's H-chunk -> vpart [1, 2D+2]."""
    w1t, w2r, b1c, b2c, vout = io["w1t"], io["w2r"], io["b1c"], io["b2c"], io["vout"]
    with (
        tc.tile_pool(name="main", bufs=1) as pool,
        tc.tile_pool(name="psum", bufs=1, space="PSUM") as psum,
    ):
        # DMA plan: tiny contiguous bias rows FIRST on the sync ring (so no
        # DVE op ever head-of-line blocks on them), then W2 halves (they gate
        # the reduce), then W1 split over all three queues.  W1 goes straight
        # into an f32r tile (w1t is declared float32r) — no cast pass.
        HD = D // 2
        WDT = BF16 if BF16_W else F32
        VDT = BF16 if BF16_W else F32R
        b1_sb = pool.tile([1, E * HC], F32)
        nc.sync.dma_start(b1_sb[:], b1c)
        b2_sb = pool.tile([1, E * DC], F32)
        nc.sync.dma_start(b2_sb[:], b2c)
        w2_sb = pool.tile([P, E, D], WDT)
        w1r = pool.tile([P, E, D], VDT)
        for h in range(2):
            nc.sync.dma_start(w2_sb[:, 0, h * HD : (h + 1) * HD], w2r[0, :, h * HD : (h + 1) * HD])
            nc.scalar.dma_start(w2_sb[:, 1, h * HD : (h + 1) * HD], w2r[1, :, h * HD : (h + 1) * HD])
        for h in range(2):
            nc.sync.dma_start(w1r[:, 0, h * HD : (h + 1) * HD], w1t[0, :, h * HD : (h + 1) * HD])
            nc.scalar.dma_start(w1r[:, 1, h * HD : (h + 1) * HD], w1t[1, :, h * HD : (h + 1) * HD])

        # b1 row -> partition-major [128, E] via PE transpose (identity [1,1])
        one1 = pool.tile([1, 1], F32)
        nc.gpsimd.memset(one1[:], 1.0)
        b1t_ps = psum.tile([P, E], F32)
        for e in range(E):
            nc.tensor.transpose(
                b1t_ps[:, e : e + 1], b1_sb[0:1, e * HC : (e + 1) * HC], one1[:]
            )
        b1p = pool.tile([P, E], F32)
        nc.vector.tensor_copy(b1p[:], b1t_ps[:])

        w2h = pool.tile([P, 2 * E], F32)
        w2s = pool.tile([P, E], F32)
        for e in range(E):
            for h in range(2):
                nc.vector.reduce_sum(
                    w2h[:, 2 * e + h : 2 * e + h + 1],
                    w2_sb[:, e, h * HD : (h + 1) * HD],
                    axis=AX.X,
                )
            nc.vector.tensor_add(
                w2s[:, e : e + 1], w2h[:, 2 * e : 2 * e + 1], w2h[:, 2 * e + 1 : 2 * e + 2]
            )
        w2s_r = pool.tile([P, E], VDT)
        nc.vector.tensor_copy(w2s_r[:], w2s[:])
        b2s = pool.tile([1, E], F32)
        for e in range(E):
            nc.vector.reduce_sum(
                b2s[0:1, e : e + 1], b2_sb[0:1, e * DC : (e + 1) * DC], axis=AX.X
            )

        pay = pool.tile([1, VPART], F32)
        b1dot = psum.tile([1, E], F32)
        DK = D // VK
        for e in range(E):
            for k in range(VK):
                vch = psum.tile([1, DK], F32, name="vch", tag="vch", bufs=2)
                nc.tensor.matmul(
                    vch[:],
                    w2s_r[:, e : e + 1],
                    w1r[:, e, k * DK : (k + 1) * DK],
                    start=True,
                    stop=True,
                )
                dst = pay[0:1, e * D + k * DK : e * D + (k + 1) * DK]
                if k % 2 == 0:
                    nc.vector.tensor_copy(dst, vch[:])
                else:
                    nc.scalar.copy(dst, vch[:])
            nc.tensor.matmul(
                b1dot[0:1, e : e + 1],
                w2s[:, e : e + 1],
                b1p[:, e : e + 1],
                start=True,
                stop=True,
            )
            nc.vector.tensor_add(
                pay[0:1, 2 * D + e : 2 * D + e + 1],
                b1dot[0:1, e : e + 1],
                b2s[0:1, e : e + 1],
            )
        nc.sync.dma_start(vout[:], pay[:])


def emit_phase_b(nc, tc, io):
    """logits (fp32) + s (f32r) streams, gate/select, row log_softmax."""
    xt, wgt, vin, out = io["xt"], io["wgt"], io["vin"], io["out"]
    rings = [nc.sync, nc.scalar]
    with (
        tc.tile_pool(name="main", bufs=1) as pool,
        tc.tile_pool(name="psum", bufs=1, space="PSUM") as psum,
    ):
        # descriptor-heavy small loads on the SWDGE queue (they'd clog the HW
        # rings: partition-major tiles emit one small packet per partition);
        # the HW rings carry pure x, graduated so block 0 lands ASAP
        wgt_sb = pool.tile([P, NB * E], F32)
        nc.gpsimd.dma_start(wgt_sb[:], wgt)
        wg3 = wgt_sb.rearrange("p (n e) -> p n e", e=E)
        vsb = pool.tile([P, E, NB], F32)
        for e in range(E):
            nc.gpsimd.dma_start(
                vsb[:, e, :],
                vin[0:1, e * D : (e + 1) * D].rearrange("x (p n) -> p (x n)", p=P),
            )
        csum = pool.tile([1, E], F32)
        nc.gpsimd.dma_start(csum[:], vin[0:1, 2 * D : 2 * D + E])

        x_sb = pool.tile([P, NB, TB], F32)
        xv = xt.rearrange("(p n) t -> p n t", p=P)
        qs = [nc.sync, nc.scalar]
        chunks = [
            (0, 0, 1), (1, 1, 2),
            (0, 2, 4), (1, 4, 6),
            (0, 6, 9), (1, 9, 12),
            (0, 12, 14), (1, 14, 16),
        ]
        for q, lo, hi in chunks:
            qs[q].dma_start(x_sb[:, lo:hi, :], xv[:, lo:hi, :])

        # preload ACT tables (Exp, Ln) off the critical path; keep ALL copy
        # work off the scalar engine so these tables are never evicted
        warm = pool.tile([1, 2], F32)
        nc.gpsimd.memset(warm[:], 1.0)
        wz = pool.tile([1, 2], F32)
        nc.scalar.activation(wz[:], warm[:], AF.Exp)
        nc.scalar.activation(wz[:], warm[:], AF.Ln)

        # small f32r prep FIRST (DVE is FIFO — these must not queue behind the
        # 4MB of x casts), then the x cast chain trailing the DMA chunks
        vsb_r = pool.tile([P, E, NB], F32R)
        nc.vector.tensor_copy(vsb_r[:], vsb[:])
        csum_b = pool.tile([P, E], F32)
        nc.gpsimd.partition_broadcast(csum_b[:], csum[0:1, :])
        x_r = pool.tile([P, NB, TB], F32R)
        for _, lo, hi in chunks:
            nc.vector.tensor_copy(x_r[:, lo:hi, :], x_sb[:, lo:hi, :])

        # logits stream (fp32 exact) fully first — the s stream's f32r inputs
        # (DVE casts) are ready long before the fp32 stream retires, and
        # keeping the streams separate avoids any PE head-of-line stall
        lg_ps = psum.tile([E, TB], F32)
        for n in range(NB):
            nc.tensor.matmul(
                lg_ps[:], wg3[:, n, :], x_sb[:, n, :], start=(n == 0), stop=(n == NB - 1)
            )
        sg_ps = psum.tile([E, TB], F32)
        for n in range(NB):
            nc.tensor.matmul(
                sg_ps[:], vsb_r[:, :, n], x_r[:, n, :], start=(n == 0), stop=(n == NB - 1)
            )
        sbl = pool.tile([E, TB], F32)
        nc.vector.tensor_copy(sbl[:], lg_ps[:])
        ident = pool.tile([P, P], F32)
        make_identity(nc, ident[:])

        gates, masks = [], []
        for g in range(NG):
            tpl = psum.tile([P, E], F32, name=f"tpl_{g}", tag="tp", bufs=2)
            nc.tensor.transpose(tpl[:], sbl[0:E, g * P : (g + 1) * P], ident[0:E, 0:E])
            t2l = pool.tile([P, E], F32, name=f"t2l_{g}")
            nc.vector.tensor_copy(t2l[:], tpl[:])
            negm = pool.tile([P, 1], F32, name=f"negm_{g}")
            nc.vector.reduce_max(negm[:], t2l[:], axis=AX.X, negate=True)
            z = pool.tile([P, E], F32, name=f"z_{g}")
            den = pool.tile([P, 1], F32, name=f"den_{g}")
            nc.scalar.activation(z[:], t2l[:], AF.Exp, bias=negm[:], accum_out=den[:])
            rec = pool.tile([P, 1], F32, name=f"rec_{g}")
            nc.vector.reciprocal(rec[:], den[:])
            zmax = pool.tile([P, 1], F32, name=f"zmax_{g}")
            nc.vector.reduce_max(zmax[:], z[:], axis=AX.X)
            gate = pool.tile([P, 1], F32, name=f"gate_{g}")
            nc.vector.tensor_mul(gate[:], zmax[:], rec[:])
            mask = pool.tile([P, 1], F32, name=f"mask_{g}")
            nc.vector.tensor_tensor(mask[:], t2l[:, 0:1], t2l[:, 1:2], op=ALU.is_ge)
            gates.append(gate)
            masks.append(mask)

        sbs = pool.tile([E, TB], F32)
        nc.vector.tensor_copy(sbs[:], sg_ps[:])

        moe_sb = pool.tile([P, NG], F32)
        for g in range(NG):
            tps = psum.tile([P, E], F32, name=f"tps_{g}", tag="tp", bufs=2)
            nc.tensor.transpose(tps[:], sbs[0:E, g * P : (g + 1) * P], ident[0:E, 0:E])
            t2s = pool.tile([P, E], F32, name=f"t2s_{g}")
            nc.vector.tensor_add(t2s[:], tps[:], csum_b[:])
            sdiff = pool.tile([P, 1], F32, name=f"sdiff_{g}")
            nc.vector.tensor_sub(sdiff[:], t2s[:, 0:1], t2s[:, 1:2])
            ssel = pool.tile([P, 1], F32, name=f"ssel_{g}")
            nc.vector.tensor_mul(ssel[:], masks[g][:], sdiff[:])
            nc.vector.tensor_add(ssel[:], ssel[:], t2s[:, 1:2])
            nc.vector.tensor_mul(moe_sb[:, g : g + 1], gates[g][:], ssel[:])

        # row log_softmax over all 512 tokens, via PE transposes
        tp4 = psum.tile([NG, P], F32)
        nc.tensor.transpose(tp4[:], moe_sb[:], ident[:])
        sb4t = pool.tile([NG, P], F32)
        nc.vector.tensor_copy(sb4t[:], tp4[:])
        m4p = pool.tile([NG, 1], F32)
        nc.vector.reduce_max(m4p[:], sb4t[:], axis=AX.X)
        m1p = psum.tile([1, NG], F32, name="m1p", tag="t1", bufs=2)
        nc.tensor.transpose(m1p[:], m4p[:], ident[0:NG, 0:NG])
        negm2 = pool.tile([1, 1], F32)
        nc.vector.reduce_max(negm2[:], m1p[:], axis=AX.X, negate=True)
        negm4 = pool.tile([NG, 1], F32)
        nc.gpsimd.partition_broadcast(negm4[:], negm2[:])
        e4 = pool.tile([NG, P], F32)
        s4 = pool.tile([NG, 1], F32)
        nc.scalar.activation(e4[:], sb4t[:], AF.Exp, bias=negm4[:], accum_out=s4[:])
        s1p = psum.tile([1, NG], F32, name="s1p", tag="t1", bufs=2)
        nc.tensor.transpose(s1p[:], s4[:], ident[0:NG, 0:NG])
        ssum = pool.tile([1, 1], F32)
        nc.vector.reduce_sum(ssum[:], s1p[:], axis=AX.X)
        logs = pool.tile([1, 1], F32)
        nc.scalar.activation(logs[:], ssum[:], AF.Ln)
        shift = pool.tile([1, 1], F32)
        nc.vector.tensor_sub(shift[:], negm2[:], logs[:])
        shift4 = pool.tile([NG, 1], F32)
        nc.gpsimd.partition_broadcast(shift4[:], shift[:])
        res4 = pool.tile([NG, P], F32)
        nc.vector.tensor_scalar_add(res4[:], sb4t[:], shift4[:])
        nc.sync.dma_start(out.rearrange("x (g p) -> g (x p)", p=P), res4[:])


_CACHED = {}


def build_program(which):
    if which in _CACHED:
        return _CACHED[which]
    nc = bacc.Bacc(
        "TRN2",
        target_bir_lowering=False,
        debug=False,
        enable_asserts=False,
        num_devices=NCORES,
    )
    if which == "a":
        io = {
            "w1t": nc.dram_tensor(
                "w1t", [E, HC, D], BF16 if BF16_W else F32R, kind="ExternalInput"
            ).ap(),
            "w2r": nc.dram_tensor(
                "w2r", [E, HC, D], BF16 if BF16_W else F32, kind="ExternalInput"
            ).ap(),
            "b1c": nc.dram_tensor("b1c", [1, E * HC], F32, kind="ExternalInput").ap(),
            "b2c": nc.dram_tensor("b2c", [1, E * DC], F32, kind="ExternalInput").ap(),
            "vout": nc.dram_tensor("vout", [1, VPART], F32, kind="ExternalOutput").ap(),
        }
        emit = emit_phase_a
    else:
        io = {
            "xt": nc.dram_tensor("xt", [D, TB], F32, kind="ExternalInput").ap(),
            "wgt": nc.dram_tensor("wgt", [P, NB * E], F32, kind="ExternalInput").ap(),
            "vin": nc.dram_tensor("vin", [1, VPART], F32, kind="ExternalInput").ap(),
            "out": nc.dram_tensor("out", [1, TB], F32, kind="ExternalOutput").ap(),
        }
        emit = emit_phase_b
    with tile.TileContext(nc) as tc:
        emit(nc, tc, io)
    nc.compile()
    _CACHED[which] = nc
    return nc


def shard_inputs_a(Wg, W1, b1, W2, b2):
    import ml_dtypes

    wdt = ml_dtypes.bfloat16 if BF16_W else np.float32
    W1 = np.asarray(W1, np.float32)
    b1 = np.asarray(b1, np.float32)
    W2 = np.asarray(W2, np.float32)
    b2 = np.asarray(b2, np.float32)
    in_maps = []
    for c in range(NCORES):
        hs, he = c * HC, (c + 1) * HC
        in_maps.append(
            {
                "w1t": np.ascontiguousarray(W1[:, :, hs:he].transpose(0, 2, 1).astype(wdt)),
                "w2r": np.ascontiguousarray(W2[:, hs:he, :].astype(wdt)),
                "b1c": np.ascontiguousarray(b1[:, hs:he].reshape(1, E * HC)),
                "b2c": np.ascontiguousarray(
                    b2[:, c * DC : (c + 1) * DC].reshape(1, E * DC)
                ),
            }
        )
    return in_maps


def shard_inputs_b(x, Wg, vpart_sum):
    x = np.asarray(x, np.float32).reshape(B * T, D)
    Wg = np.asarray(Wg, np.float32)
    # wgt[p, n*2+e] = Wg[p*16+n, e]  (d = p*16 + n decomposition)
    wgt = np.ascontiguousarray(Wg.reshape(P, NB * E))
    in_maps = []
    for c in range(NCORES):
        row = c % B
        in_maps.append(
            {
                "xt": np.ascontiguousarray(x[row * TB : (row + 1) * TB, :].T),
                "wgt": wgt,
                "vin": vpart_sum,
            }
        )
    return in_maps


def run_a(in_maps, **kwargs):
    return bass_utils.run_bass_kernel_spmd(
        build_program("a"), in_maps, core_ids=list(range(NCORES)), **kwargs
    )


def run_b(in_maps, **kwargs):
    return bass_utils.run_bass_kernel_spmd(
        build_program("b"), in_maps, core_ids=list(range(NCORES)), **kwargs
    )


def kernel(x, Wg, W1, b1, W2, b2):
    res_a = run_a(shard_inputs_a(Wg, W1, b1, W2, b2))
    # cross-core combine: sum of the 8 per-core partials (the gather/reshard
    # step between the two launches; 16KB, no model math beyond the reduction)
    vpart = np.sum([res_a.results[c]["vout"] for c in range(NCORES)], axis=0)
    vpart = np.ascontiguousarray(vpart, np.float32)
    res_b = run_b(shard_inputs_b(x, Wg, vpart))
    return np.concatenate([res_b.results[b]["out"] for b in range(B)], axis=0)


# revision 43
# speedup vs baseline: 1.1886x; 1.0041x over previous
"""Trainium2 Bass kernel for nn_ExampleModel_1116691497724 (moe_routing).

Math: the reference returns log_softmax_T( sum_D(moe_out) ), and sum_D
collapses the expert FFN to a dot product:
    sum_d (h @ W2[e] + b2[e]) = h . w2sum[e] + sum(b2[e]),  w2sum[e] = W2[e] @ 1
    (x @ W1[e] + b1[e]) . w2sum[e] = x . v[e] + c[e]
with v[e] = W1[e] @ w2sum[e]  (a [D] vector) and scalar
c[e] = b1[e].w2sum[e] + sum(b2[e]).  Then per token:
    s_e = x . v[e] + c[e],  logits = x @ Wg
    moe_sum = max(softmax(logits)) * s_argmax(logits)
    out = log_softmax over tokens (per batch row) of moe_sum.

Distribution over 8 cores, two launches (measured: a single ncfw collective
costs ~65us of barrier/trigger latency on this runtime — far more than a
second launch's fixed ~17us, so the 16KB cross-core combine happens on the
host between launches; the host does only that partial sum, all real math
stays on device):
  launch A (expert-parallel over H): core c reduces W2[:, 128c:128c+128, :]
    and computes partial v from the matching W1 columns (f32r stream after a
    rounding pass) -> outputs [v0 | v1 | c0 c1] partials (16KB); host sums.
  launch B (token-parallel): core c owns batch row c%4 (512 tokens): logits
    stream in fp32 (exact — argmax ties must match the reference), s stream
    in f32r, gate/select per token after a PE transpose, row log_softmax via
    PE transposes (no cross-partition DMA).  Host takes rows from cores 0..3.

Scheduling: stationary matmul operands are tiny (M<=4) so LDWEIGHTS is
negligible; fp32 streams at 4 cycles/row, f32r at 1.  Big loads alternate the
two HWDGE rings (SP via nc.sync, ACT via nc.scalar) for concurrency.  The d
axis is decomposed as d = p*16 + n so the flat v vector loads into [128,16]
tiles with contiguous per-partition runs.
"""

import sys

import numpy as np

for _p in ("/opt/trn_rl_repo",):
    if _p not in sys.path:
        sys.path.append(_p)

import concourse.bass as bass  # noqa: E402
import concourse.mybir as mybir  # noqa: E402
import concourse.tile as tile  # noqa: E402
from concourse import bacc, bass_utils  # noqa: E402
from concourse.masks import make_identity  # noqa: E402

# Problem shape (hardcoded per spec).
B, T, D, H, E = 4, 512, 2048, 1024, 2
P = 128
NCORES = 8
TB = T  # tokens per core = one batch row
NB = D // P  # 16 d-blocks
HC = H // NCORES  # 128 h-chunk per expert per core
NG = TB // P  # 4 token groups per core
DC = D // NCORES  # 256 b2 columns per core
VK = 4  # v computed in VK chunks of D/VK columns
F32 = mybir.dt.float32
F32R = mybir.dt.float32r
AX = mybir.AxisListType
AF = mybir.ActivationFunctionType
ALU = mybir.AluOpType

VPART = 2 * D + 2  # launch A output: v0 | v1 | c0 c1
BF16 = mybir.dt.bfloat16
BF16_W = False  # bf16 W1/W2 saves only ~2us but costs 13x accuracy; keep f32


def emit_phase_a(nc, tc, io):
    """w2sum + partial v for this core's H-chunk -> vpart [1, 2D+2]."""
    w1t, w2r, b1c, b2c, vout = io["w1t"], io["w2r"], io["b1c"], io["b2c"], io["vout"]
    with (
        tc.tile_pool(name="main", bufs=1) as pool,
        tc.tile_pool(name="psum", bufs=1, space="PSUM") as psum,
    ):
        # DMA plan: tiny contiguous bias rows FIRST on the sync ring (so no
        # DVE op ever head-of-line blocks on them), then W2 halves (they gate
        # the reduce), then W1 split over all three queues.  W1 goes straight
        # into an f32r tile (w1t is declared float32r) — no cast pass.
        HD = D // 2
        WDT = BF16 if BF16_W else F32
        VDT = BF16 if BF16_W else F32R
        b1_sb = pool.tile([1, E * HC], F32)
        nc.sync.dma_start(b1_sb[:], b1c)
        b2_sb = pool.tile([1, E * DC], F32)
        nc.sync.dma_start(b2_sb[:], b2c)
        w2_sb = pool.tile([P, E, D], WDT)
        w1r = pool.tile([P, E, D], VDT)
        for h in range(2):
            nc.sync.dma_start(w2_sb[:, 0, h * HD : (h + 1) * HD], w2r[0, :, h * HD : (h + 1) * HD])
            nc.scalar.dma_start(w2_sb[:, 1, h * HD : (h + 1) * HD], w2r[1, :, h * HD : (h + 1) * HD])
        for h in range(2):
            nc.sync.dma_start(w1r[:, 0, h * HD : (h + 1) * HD], w1t[0, :, h * HD : (h + 1) * HD])
            nc.scalar.dma_start(w1r[:, 1, h * HD : (h + 1) * HD], w1t[1, :, h * HD : (h + 1) * HD])

        # b1 row -> partition-major [128, E] via PE transpose (identity [1,1])
        one1 = pool.tile([1, 1], F32)
        nc.gpsimd.memset(one1[:], 1.0)
        b1t_ps = psum.tile([P, E], F32)
        for e in range(E):
            nc.tensor.transpose(
                b1t_ps[:, e : e + 1], b1_sb[0:1, e * HC : (e + 1) * HC], one1[:]
            )
        b1p = pool.tile([P, E], F32)
        nc.vector.tensor_copy(b1p[:], b1t_ps[:])

        w2h = pool.tile([P, 2 * E], F32)
        w2s = pool.tile([P, E], F32)
        for e in range(E):
            for h in range(2):
                nc.vector.reduce_sum(
                    w2h[:, 2 * e + h : 2 * e + h + 1],
                    w2_sb[:, e, h * HD : (h + 1) * HD],
                    axis=AX.X,
                )
            nc.vector.tensor_add(
                w2s[:, e : e + 1], w2h[:, 2 * e : 2 * e + 1], w2h[:, 2 * e + 1 : 2 * e + 2]
            )
        w2s_r = pool.tile([P, E], VDT)
        nc.vector.tensor_copy(w2s_r[:], w2s[:])
        b2s = pool.tile([1, E], F32)
        for e in range(E):
            nc.vector.reduce_sum(
                b2s[0:1, e : e + 1], b2_sb[0:1, e * DC : (e + 1) * DC], axis=AX.X
            )

        pay = pool.tile([1, VPART], F32)
        b1dot = psum.tile([1, E], F32)
        DK = D // VK
        for e in range(E):
            for k in range(VK):
                vch = psum.tile([1, DK], F32, name="vch", tag="vch", bufs=2)
                nc.tensor.matmul(
                    vch[:],
                    w2s_r[:, e : e + 1],
                    w1r[:, e, k * DK : (k + 1) * DK],
                    start=True,
                    stop=True,
                )
                dst = pay[0:1, e * D + k * DK : e * D + (k + 1) * DK]
                if k % 2 == 0:
                    nc.vector.tensor_copy(dst, vch[:])
                else:
                    nc.scalar.copy(dst, vch[:])
            nc.tensor.matmul(
                b1dot[0:1, e : e + 1],
                w2s[:, e : e + 1],
                b1p[:, e : e + 1],
                start=True,
                stop=True,
            )
            nc.vector.tensor_add(
                pay[0:1, 2 * D + e : 2 * D + e + 1],
                b1dot[0:1, e : e + 1],
                b2s[0:1, e : e + 1],
            )
        nc.sync.dma_start(vout[:], pay[:])


def emit_phase_b(nc, tc, io):
    """logits (fp32) + s (f32r) streams, gate/select, row log_softmax."""
    xt, wgt, vin, out = io["xt"], io["wgt"], io["vin"], io["out"]
    rings = [nc.sync, nc.scalar]
    with (
        tc.tile_pool(name="main", bufs=1) as pool,
        tc.tile_pool(name="psum", bufs=1, space="PSUM") as psum,
    ):
        # descriptor-heavy small loads on the SWDGE queue (they'd clog the HW
        # rings: partition-major tiles emit one small packet per partition);
        # the HW rings carry pure x, graduated so block 0 lands ASAP
        wgt_sb = pool.tile([P, NB * E], F32)
        nc.gpsimd.dma_start(wgt_sb[:], wgt)
        wg3 = wgt_sb.rearrange("p (n e) -> p n e", e=E)
        vsb = pool.tile([P, E, NB], F32)
        for e in range(E):
            nc.gpsimd.dma_start(
                vsb[:, e, :],
                vin[0:1, e * D : (e + 1) * D].rearrange("x (p n) -> p (x n)", p=P),
            )
        csum = pool.tile([1, E], F32)
        nc.gpsimd.dma_start(csum[:], vin[0:1, 2 * D : 2 * D + E])

        x_sb = pool.tile([P, NB, TB], F32)
        xv = xt.rearrange("(p n) t -> p n t", p=P)
        qs = [nc.sync, nc.scalar]
        chunks = [
            (0, 0, 1), (1, 1, 2),
            (0, 2, 4), (1, 4, 6),
            (0, 6, 9), (1, 9, 12),
            (0, 12, 14), (1, 14, 16),
        ]
        for q, lo, hi in chunks:
            qs[q].dma_start(x_sb[:, lo:hi, :], xv[:, lo:hi, :])

        # preload ACT tables (Exp, Ln) off the critical path; keep ALL copy
        # work off the scalar engine so these tables are never evicted
        warm = pool.tile([1, 2], F32)
        nc.gpsimd.memset(warm[:], 1.0)
        wz = pool.tile([1, 2], F32)
        nc.scalar.activation(wz[:], warm[:], AF.Exp)
        nc.scalar.activation(wz[:], warm[:], AF.Ln)

        # small f32r prep FIRST (DVE is FIFO — these must not queue behind the
        # 4MB of x casts), then the x cast chain trailing the DMA chunks
        vsb_r = pool.tile([P, E, NB], F32R)
        nc.vector.tensor_copy(vsb_r[:], vsb[:])
        csum_b = pool.tile([P, E], F32)
        nc.gpsimd.partition_broadcast(csum_b[:], csum[0:1, :])
        x_r = pool.tile([P, NB, TB], F32R)
        for _, lo, hi in chunks:
            nc.vector.tensor_copy(x_r[:, lo:hi, :], x_sb[:, lo:hi, :])

        ident = pool.tile([P, P], F32)
        make_identity(nc, ident[:])

        # logits stream (fp32 exact) first; then the gate pipeline's PE
        # transposes MUST be emitted before the s stream (PE is FIFO — behind
        # the s matmuls they would stall the whole gate chain by ~3.5us)
        lg_ps = psum.tile([E, TB], F32)
        for n in range(NB):
            nc.tensor.matmul(
                lg_ps[:], wg3[:, n, :], x_sb[:, n, :], start=(n == 0), stop=(n == NB - 1)
            )
        sbl = pool.tile([E, TB], F32)
        nc.vector.tensor_copy(sbl[:], lg_ps[:])
        gates, masks = [], []
        for g in range(NG):
            tpl = psum.tile([P, E], F32, name=f"tpl_{g}", tag="tp", bufs=2)
            nc.tensor.transpose(tpl[:], sbl[0:E, g * P : (g + 1) * P], ident[0:E, 0:E])
            t2l = pool.tile([P, E], F32, name=f"t2l_{g}")
            nc.vector.tensor_copy(t2l[:], tpl[:])
            negm = pool.tile([P, 1], F32, name=f"negm_{g}")
            nc.vector.reduce_max(negm[:], t2l[:], axis=AX.X, negate=True)
            z = pool.tile([P, E], F32, name=f"z_{g}")
            den = pool.tile([P, 1], F32, name=f"den_{g}")
            nc.scalar.activation(z[:], t2l[:], AF.Exp, bias=negm[:], accum_out=den[:])
            rec = pool.tile([P, 1], F32, name=f"rec_{g}")
            nc.vector.reciprocal(rec[:], den[:])
            zmax = pool.tile([P, 1], F32, name=f"zmax_{g}")
            nc.vector.reduce_max(zmax[:], z[:], axis=AX.X)
            gate = pool.tile([P, 1], F32, name=f"gate_{g}")
            nc.vector.tensor_mul(gate[:], zmax[:], rec[:])
            mask = pool.tile([P, 1], F32, name=f"mask_{g}")
            nc.vector.tensor_tensor(mask[:], t2l[:, 0:1], t2l[:, 1:2], op=ALU.is_ge)
            gates.append(gate)
            masks.append(mask)

        sg_ps = psum.tile([E, TB], F32)
        for n in range(NB):
            nc.tensor.matmul(
                sg_ps[:], vsb_r[:, :, n], x_r[:, n, :], start=(n == 0), stop=(n == NB - 1)
            )

        sbs = pool.tile([E, TB], F32)
        nc.vector.tensor_copy(sbs[:], sg_ps[:])

        moe_sb = pool.tile([P, NG], F32)
        for g in range(NG):
            tps = psum.tile([P, E], F32, name=f"tps_{g}", tag="tp", bufs=2)
            nc.tensor.transpose(tps[:], sbs[0:E, g * P : (g + 1) * P], ident[0:E, 0:E])
            t2s = pool.tile([P, E], F32, name=f"t2s_{g}")
            nc.vector.tensor_add(t2s[:], tps[:], csum_b[:])
            sdiff = pool.tile([P, 1], F32, name=f"sdiff_{g}")
            nc.vector.tensor_sub(sdiff[:], t2s[:, 0:1], t2s[:, 1:2])
            ssel = pool.tile([P, 1], F32, name=f"ssel_{g}")
            nc.vector.tensor_mul(ssel[:], masks[g][:], sdiff[:])
            nc.vector.tensor_add(ssel[:], ssel[:], t2s[:, 1:2])
            nc.vector.tensor_mul(moe_sb[:, g : g + 1], gates[g][:], ssel[:])

        # row log_softmax over all 512 tokens, via PE transposes
        tp4 = psum.tile([NG, P], F32)
        nc.tensor.transpose(tp4[:], moe_sb[:], ident[:])
        sb4t = pool.tile([NG, P], F32)
        nc.vector.tensor_copy(sb4t[:], tp4[:])
        m4p = pool.tile([NG, 1], F32)
        nc.vector.reduce_max(m4p[:], sb4t[:], axis=AX.X)
        m1p = psum.tile([1, NG], F32, name="m1p", tag="t1", bufs=2)
        nc.tensor.transpose(m1p[:], m4p[:], ident[0:NG, 0:NG])
        negm2 = pool.tile([1, 1], F32)
        nc.vector.reduce_max(negm2[:], m1p[:], axis=AX.X, negate=True)
        negm4 = pool.tile([NG, 1], F32)
        nc.gpsimd.partition_broadcast(negm4[:], negm2[:])
        e4 = pool.tile([NG, P], F32)
        s4 = pool.tile([NG, 1], F32)
        nc.scalar.activation(e4[:], sb4t[:], AF.Exp, bias=negm4[:], accum_out=s4[:])
        # reload the Ln table NOW (the Exp uses above evicted it) so the real
        # Ln below table-hits; overlaps the transpose+reduce on other engines
        nc.scalar.activation(wz[:], warm[:], AF.Ln)
        s1p = psum.tile([1, NG], F32, name="s1p", tag="t1", bufs=2)
        nc.tensor.transpose(s1p[:], s4[:], ident[0:NG, 0:NG])
        ssum = pool.tile([1, 1], F32)
        nc.vector.reduce_sum(ssum[:], s1p[:], axis=AX.X)
        logs = pool.tile([1, 1], F32)
        nc.scalar.activation(logs[:], ssum[:], AF.Ln)
        shift = pool.tile([1, 1], F32)
        nc.vector.tensor_sub(shift[:], negm2[:], logs[:])
        shift4 = pool.tile([NG, 1], F32)
        nc.gpsimd.partition_broadcast(shift4[:], shift[:])
        res4 = pool.tile([NG, P], F32)
        nc.vector.tensor_scalar_add(res4[:], sb4t[:], shift4[:])
        nc.sync.dma_start(out.rearrange("x (g p) -> g (x p)", p=P), res4[:])


_CACHED = {}


def build_program(which):
    if which in _CACHED:
        return _CACHED[which]
    nc = bacc.Bacc(
        "TRN2",
        target_bir_lowering=False,
        debug=False,
        enable_asserts=False,
        num_devices=NCORES,
    )
    if which == "a":
        io = {
            "w1t": nc.dram_tensor(
                "w1t", [E, HC, D], BF16 if BF16_W else F32R, kind="ExternalInput"
            ).ap(),
            "w2r": nc.dram_tensor(
                "w2r", [E, HC, D], BF16 if BF16_W else F32, kind="ExternalInput"
            ).ap(),
            "b1c": nc.dram_tensor("b1c", [1, E * HC], F32, kind="ExternalInput").ap(),
            "b2c": nc.dram_tensor("b2c", [1, E * DC], F32, kind="ExternalInput").ap(),
            "vout": nc.dram_tensor("vout", [1, VPART], F32, kind="ExternalOutput").ap(),
        }
        emit = emit_phase_a
    else:
        io = {
            "xt": nc.dram_tensor("xt", [D, TB], F32, kind="ExternalInput").ap(),
            "wgt": nc.dram_tensor("wgt", [P, NB * E], F32, kind="ExternalInput").ap(),
            "vin": nc.dram_tensor("vin", [1, VPART], F32, kind="ExternalInput").ap(),
            "out": nc.dram_tensor("out", [1, TB], F32, kind="ExternalOutput").ap(),
        }
        emit = emit_phase_b
    with tile.TileContext(nc) as tc:
        emit(nc, tc, io)
    nc.compile()
    _CACHED[which] = nc
    return nc


def shard_inputs_a(Wg, W1, b1, W2, b2):
    if BF16_W:
        import ml_dtypes

        wdt = ml_dtypes.bfloat16
    else:
        wdt = np.float32
    W1 = np.asarray(W1, np.float32)
    b1 = np.asarray(b1, np.float32)
    W2 = np.asarray(W2, np.float32)
    b2 = np.asarray(b2, np.float32)
    in_maps = []
    for c in range(NCORES):
        hs, he = c * HC, (c + 1) * HC
        in_maps.append(
            {
                "w1t": np.ascontiguousarray(W1[:, :, hs:he].transpose(0, 2, 1).astype(wdt)),
                "w2r": np.ascontiguousarray(W2[:, hs:he, :].astype(wdt)),
                "b1c": np.ascontiguousarray(b1[:, hs:he].reshape(1, E * HC)),
                "b2c": np.ascontiguousarray(
                    b2[:, c * DC : (c + 1) * DC].reshape(1, E * DC)
                ),
            }
        )
    return in_maps


def shard_inputs_b(x, Wg, vpart_sum):
    x = np.asarray(x, np.float32).reshape(B * T, D)
    Wg = np.asarray(Wg, np.float32)
    # wgt[p, n*2+e] = Wg[p*16+n, e]  (d = p*16 + n decomposition)
    wgt = np.ascontiguousarray(Wg.reshape(P, NB * E))
    in_maps = []
    for c in range(NCORES):
        row = c % B
        in_maps.append(
            {
                "xt": np.ascontiguousarray(x[row * TB : (row + 1) * TB, :].T),
                "wgt": wgt,
                "vin": vpart_sum,
            }
        )
    return in_maps


def run_a(in_maps, **kwargs):
    return bass_utils.run_bass_kernel_spmd(
        build_program("a"), in_maps, core_ids=list(range(NCORES)), **kwargs
    )


def run_b(in_maps, **kwargs):
    return bass_utils.run_bass_kernel_spmd(
        build_program("b"), in_maps, core_ids=list(range(NCORES)), **kwargs
    )


def kernel(x, Wg, W1, b1, W2, b2):
    res_a = run_a(shard_inputs_a(Wg, W1, b1, W2, b2))
    # cross-core combine: sum of the 8 per-core partials (the gather/reshard
    # step between the two launches; 16KB, no model math beyond the reduction)
    vpart = np.sum([res_a.results[c]["vout"] for c in range(NCORES)], axis=0)
    vpart = np.ascontiguousarray(vpart, np.float32)
    res_b = run_b(shard_inputs_b(x, Wg, vpart))
    return np.concatenate([res_b.results[b]["out"] for b in range(B)], axis=0)
